# revision 32
# baseline (speedup 1.0000x reference)
"""Trainium2 Bass kernel: gumbel-softmax-argmax embedding lookup (end-to-end).

Reference math (nn_End2End_49495203119139):
    hot  = argmax_V(softmax((logits + gumbel)/tau))       == argmax_V(logits+gumbel)
    row  = grid_sample-nearest index map of hot            == ROWMAP[hot]  (LUT)
    tok_emb = W[row][:, col_map]   with col_map == arange(E)  (verified at runtime)
    inputs_embeds = tok_emb * mask
    psg_roll = roll(psg_ids, 1, axis=1); psg_roll[:,0] = 1
    extr  = (1 - mask[:, ::-1]) * psg_roll
    trunc = rotate_right(extr, shifts) with shifts = mask.sum(-1)   (per row)
    flag  = cumsum(trunc != 0, -1) > 0
    out   = inputs_embeds + where(flag, W[trunc], 0)

Sharding: data-parallel over batch. B=16 over 8 cores -> 2 batch rows
(= 2 token tiles of 128) per core; the embedding table is replicated.

Per-core device plan (memory-bound part = streaming logits+gumbel, 66 MB,
~184 us HBM floor at ~358 GB/s per core):
  - for each token tile (128 tokens on partitions) and each vocab chunk
    [128 x 4016]: HWDGE-load the logits chunk, then add the gumbel chunk
    with two SWDGE inline-accumulate DMAs (CCE add; descriptors kept
    <= 2008 elements — the 2048-element CCE limit crashes the device).
    DVE `max` finds the chunk max, `max_index` the first within-chunk
    argmax position (ties resolve to the lowest index, matching argmax).
  - chunk winner (lowest chunk attaining the global max) + within-chunk
    index give `hot`; ROWMAP and W rows come via indirect DMA gathers.
  - the passage branch is pure index arithmetic on [128,1] tiles: the
    reverse/roll/rotate are folded into gather indices modulo L, the
    mask-sum and cumsum are 0/1 matmuls against ones/triangular matrices
    (exact in any PE precision).
Predicted 222.6 us/core by the TimelineSim cost model; a hardware
min-slope measurement gave ~204 us.
"""

import numpy as np

B = 16
L = 128
V = 32128
E = 768
N_CORES = 8
B_LOC = B // N_CORES          # batch rows per core
CH = 4016                     # vocab chunk (free dim) per streamed tile
NCH = V // CH                 # 8 chunks
NEG = -3.0e38


def _build(nc_mod, dims=None, body_reps=1):
    """Build the per-core Bass module. dims allows small smoke-test builds;
    body_reps>1 repeats the whole body (for slope-based benchmarking)."""
    import concourse.tile as tile
    from concourse import bass, mybir
    from concourse.bass import IndirectOffsetOnAxis

    d = dims or {}
    v = d.get("V", V)
    e = d.get("E", E)
    ch = d.get("CH", CH)
    nch = v // ch
    b_loc = d.get("B_LOC", B_LOC)
    rows = b_loc * L
    lbufs = d.get("LBUFS", 6)
    skip_tail = d.get("SKIP_TAIL", False)
    skip_accum = d.get("SKIP_ACCUM", False)
    skip_maxidx = d.get("SKIP_MAXIDX", False)
    tail_after_each = d.get("TAIL_AFTER_EACH", False)
    # how logits+gumbel are summed: "accum" = DMA CCE inline add (SWDGE),
    # "dve" = plain loads + DVE adds, "split" = adds alternate DVE/GpSimd
    add_mode = d.get("ADD_MODE", "accum")
    max_eng = d.get("MAX_ENG", "dve")
    smalls_on_act = d.get("SMALLS_ON_ACT", False)

    nc = nc_mod
    f32 = mybir.dt.float32
    i32 = mybir.dt.int32
    u32 = mybir.dt.uint32
    Op = mybir.AluOpType
    AX = mybir.AxisListType

    two_tables = d.get("TWO_TABLES", False)

    logits_h = nc.dram_tensor("logits", [rows, v], f32, kind="ExternalInput")
    gumbel_h = nc.dram_tensor("gumbel", [rows, v], f32, kind="ExternalInput")
    mask_h = nc.dram_tensor("mask", [rows, 1], i32, kind="ExternalInput")
    psg_h = nc.dram_tensor("psg", [rows, 1], i32, kind="ExternalInput")
    wte_h = nc.dram_tensor("wte", [v, e], f32, kind="ExternalInput")
    # the token branch reads W[:, col_map]; col_map is the identity here, so
    # both branches normally share one table (TWO_TABLES is a safety fallback)
    wtok_h = nc.dram_tensor("wte_tok", [v, e], f32, kind="ExternalInput") if two_tables else wte_h
    rowmap_h = nc.dram_tensor("rowmap", [v, 1], i32, kind="ExternalInput")
    tri_h = nc.dram_tensor("tri", [L, L], f32, kind="ExternalInput")
    out_h = nc.dram_tensor("out", [rows, e], f32, kind="ExternalOutput")
    # tiny passthrough pair so a benchmark can chain executions back-to-back
    chain_h = nc.dram_tensor("chain", [L, 8], f32, kind="ExternalInput")
    chain_o = nc.dram_tensor("chain_out", [L, 8], f32, kind="ExternalOutput")

    with tile.TileContext(nc) as tc:
        with (
            tc.tile_pool(name="lpool", bufs=lbufs) as lpool,
            tc.tile_pool(name="stats", bufs=2) as stats,
            tc.tile_pool(name="small", bufs=2) as small,
            tc.tile_pool(name="emb", bufs=2) as emb,
            tc.tile_pool(name="consts", bufs=1) as consts,
            tc.tile_pool(name="psum", bufs=2, space="PSUM") as psum,
        ):
            # ---- benchmark chain passthrough ----
            cht = consts.tile([L, 8], f32)
            nc.sync.dma_start(out=cht[:], in_=chain_h[:])
            nc.sync.dma_start(out=chain_o[:], in_=cht[:])

            # ---- per-core constants (built once) ----
            ones_mat = consts.tile([L, L], f32)
            nc.vector.memset(ones_mat[:], 1.0)
            tri_sb = consts.tile([L, L], f32)
            nc.sync.dma_start(out=tri_sb[:], in_=tri_h[:])

            iota_p_i = consts.tile([L, 1], i32)
            nc.gpsimd.iota(iota_p_i[:], pattern=[[1, 1]], base=0, channel_multiplier=1)
            iota_p = consts.tile([L, 1], f32)
            nc.vector.tensor_copy(out=iota_p[:], in_=iota_p_i[:])

            iota8_i = consts.tile([L, nch], i32)
            nc.gpsimd.iota(iota8_i[:], pattern=[[1, nch]], base=0, channel_multiplier=0)
            iota8 = consts.tile([L, nch], f32)
            nc.vector.tensor_copy(out=iota8[:], in_=iota8_i[:])
            # c8rev[c] = nch - c  (used to pick the LOWEST chunk that attains the max)
            c8rev = consts.tile([L, nch], f32)
            nc.vector.tensor_scalar(c8rev[:], iota8[:], -1.0, float(nch), op0=Op.mult, op1=Op.add)

            def psg_phase(t):
                """Everything that does not depend on the streamed logits:
                mask/psg index arithmetic, flag, psg-embedding gather."""
                tok = slice(t * L, (t + 1) * L)
                mask_i = small.tile([L, 1], i32, tag="mask_i")
                nc.sync.dma_start(out=mask_i[:], in_=mask_h[tok, :])
                mask_f = small.tile([L, 1], f32, tag="mask_f")
                nc.vector.tensor_copy(out=mask_f[:], in_=mask_i[:])

                # s (broadcast to all partitions) = sum(mask) via ones matmul
                s_ps = psum.tile([L, 1], f32, tag="s_ps")
                nc.tensor.matmul(out=s_ps[:], lhsT=ones_mat[:], rhs=mask_f[:], start=True, stop=True)
                s_bc = small.tile([L, 1], f32, tag="s_bc")
                nc.vector.tensor_copy(out=s_bc[:], in_=s_ps[:])

                def mod_l(x_ap, lo_fix=True, hi_fix=True, tagp=""):
                    # x <- x mod L for x in (-L, 2L)
                    if hi_fix:
                        ge = small.tile([L, 1], f32, tag="ge" + tagp)
                        nc.vector.tensor_scalar(ge[:], x_ap, float(L), None, op0=Op.is_ge)
                        nc.vector.scalar_tensor_tensor(
                            out=x_ap, in0=ge[:], scalar=-float(L), in1=x_ap, op0=Op.mult, op1=Op.add)
                    if lo_fix:
                        lt_ = small.tile([L, 1], f32, tag="lt" + tagp)
                        nc.vector.tensor_scalar(lt_[:], x_ap, 0.0, None, op0=Op.is_lt)
                        nc.vector.scalar_tensor_tensor(
                            out=x_ap, in0=lt_[:], scalar=float(L), in1=x_ap, op0=Op.mult, op1=Op.add)

                # fidx = (L-1 + s - l) mod L   (flipped-mask gather index)
                fidx = small.tile([L, 1], f32, tag="fidx")
                nc.vector.scalar_tensor_tensor(
                    out=fidx[:], in0=s_bc[:], scalar=float(L - 1), in1=iota_p[:],
                    op0=Op.add, op1=Op.subtract)
                mod_l(fidx[:], lo_fix=False, tagp="f")

                # pidx = (L-1 - s + l) mod L   (rolled-psg gather index)
                pidx = small.tile([L, 1], f32, tag="pidx")
                nc.vector.scalar_tensor_tensor(
                    out=pidx[:], in0=s_bc[:], scalar=-1.0, in1=iota_p[:],
                    op0=Op.mult, op1=Op.add)
                nc.vector.tensor_scalar(pidx[:], pidx[:], float(L - 1), None, op0=Op.add)
                mod_l(pidx[:], tagp="p")

                # k = (l - s) mod L ; BOS position is k == 0
                kk = small.tile([L, 1], f32, tag="kk")
                nc.vector.scalar_tensor_tensor(
                    out=kk[:], in0=s_bc[:], scalar=-1.0, in1=iota_p[:],
                    op0=Op.mult, op1=Op.add)
                mod_l(kk[:], hi_fix=False, tagp="k")
                bos = small.tile([L, 1], f32, tag="bos")
                nc.vector.tensor_scalar(bos[:], kk[:], 0.0, None, op0=Op.is_equal)

                # gather mask[fidx] and psg[pidx] (within this batch row)
                fr_i = small.tile([L, 1], i32, tag="fr_i")
                nc.vector.tensor_scalar(fidx[:], fidx[:], float(t * L), None, op0=Op.add)
                nc.vector.tensor_copy(out=fr_i[:], in_=fidx[:])
                mrev = small.tile([L, 1], i32, tag="mrev")
                nc.gpsimd.indirect_dma_start(
                    out=mrev[:], out_offset=None, in_=mask_h[:],
                    in_offset=IndirectOffsetOnAxis(ap=fr_i[:, 0:1], axis=0),
                )
                pr_i = small.tile([L, 1], i32, tag="pr_i")
                nc.vector.tensor_scalar(pidx[:], pidx[:], float(t * L), None, op0=Op.add)
                nc.vector.tensor_copy(out=pr_i[:], in_=pidx[:])
                prot = small.tile([L, 1], i32, tag="prot")
                nc.gpsimd.indirect_dma_start(
                    out=prot[:], out_offset=None, in_=psg_h[:],
                    in_offset=IndirectOffsetOnAxis(ap=pr_i[:, 0:1], axis=0),
                )

                # f_rot = 1 - mask[fidx]
                mrev_f = small.tile([L, 1], f32, tag="mrev_f")
                nc.vector.tensor_copy(out=mrev_f[:], in_=mrev[:])
                frot = small.tile([L, 1], f32, tag="frot")
                nc.vector.tensor_scalar(frot[:], mrev_f[:], -1.0, 1.0, op0=Op.mult, op1=Op.add)
                # psg_rot = bos ? 1 : psg[pidx]
                prot_f = small.tile([L, 1], f32, tag="prot_f")
                nc.vector.tensor_copy(out=prot_f[:], in_=prot[:])
                nbos = small.tile([L, 1], f32, tag="nbos")
                nc.vector.tensor_scalar(nbos[:], bos[:], -1.0, 1.0, op0=Op.mult, op1=Op.add)
                nc.vector.tensor_tensor(out=prot_f[:], in0=prot_f[:], in1=nbos[:], op=Op.mult)
                nc.vector.tensor_tensor(out=prot_f[:], in0=prot_f[:], in1=bos[:], op=Op.add)
                # trunc = f_rot * psg_rot
                trunc = small.tile([L, 1], f32, tag="trunc")
                nc.vector.tensor_tensor(out=trunc[:], in0=frot[:], in1=prot_f[:], op=Op.mult)

                # flag = cumsum(trunc != 0) > 0 via triangular matmul
                nz = small.tile([L, 1], f32, tag="nz")
                nc.vector.tensor_scalar(nz[:], trunc[:], 0.0, None, op0=Op.not_equal)
                cnt_ps = psum.tile([L, 1], f32, tag="cnt_ps")
                nc.tensor.matmul(out=cnt_ps[:], lhsT=tri_sb[:], rhs=nz[:], start=True, stop=True)
                flag = small.tile([L, 1], f32, tag="flag")
                nc.vector.tensor_scalar(flag[:], cnt_ps[:], 0.0, None, op0=Op.is_gt)

                trunc_i = small.tile([L, 1], i32, tag="trunc_i")
                nc.vector.tensor_copy(out=trunc_i[:], in_=trunc[:])
                psgemb = emb.tile([L, e], f32, tag="psgemb")
                nc.gpsimd.indirect_dma_start(
                    out=psgemb[:], out_offset=None, in_=wte_h[:],
                    in_offset=IndirectOffsetOnAxis(ap=trunc_i[:, 0:1], axis=0),
                )
                return mask_f, flag, psgemb

            def stream_phase(t):
                """DMA-bound pass over the vocab: per chunk, load logits,
                accumulate gumbel in the DMA datapath, track max + argmax."""
                tok = slice(t * L, (t + 1) * L)
                m_all = stats.tile([L, nch], f32, tag="m_all")
                idx_all = stats.tile([L, nch], f32, tag="idx_all")
                for c in range(nch):
                    lt = lpool.tile([L, ch], f32, tag="lt")
                    lo = c * ch
                    nc.sync.dma_start(out=lt[:], in_=logits_h[tok, lo:lo + ch])
                    if add_mode == "accum":
                        # s = logits + gumbel via DMA CCE inline add;
                        # descriptors must stay <= 2048 elements each.
                        half = ch // 2
                        if not skip_accum:
                            if d.get("ACCUM3D", False):
                                gsrc = gumbel_h[tok, lo:lo + ch].rearrange(
                                    "p (a b) -> p a b", b=half)
                                ldst = lt[:].rearrange("p (a b) -> p a b", b=half)
                                nc.gpsimd.dma_start(out=ldst, in_=gsrc, accum_op=Op.add)
                            else:
                                nc.gpsimd.dma_start(
                                    out=lt[:, 0:half], in_=gumbel_h[tok, lo:lo + half],
                                    accum_op=Op.add)
                                nc.gpsimd.dma_start(
                                    out=lt[:, half:ch], in_=gumbel_h[tok, lo + half:lo + ch],
                                    accum_op=Op.add)
                    elif add_mode == "hybrid":
                        # half the gumbel chunk via SWDGE inline-add DMA,
                        # half via HWDGE load + DVE add: balances Pool.SEQ
                        # descriptor emission against DVE cycles.
                        half = ch // 2
                        nc.gpsimd.dma_start(
                            out=lt[:, 0:half], in_=gumbel_h[tok, lo:lo + half],
                            accum_op=Op.add)
                        gt = lpool.tile([L, half], f32, tag="gt")
                        nc.sync.dma_start(out=gt[:], in_=gumbel_h[tok, lo + half:lo + ch])
                        nc.vector.tensor_tensor(out=lt[:, half:ch], in0=lt[:, half:ch], in1=gt[:], op=Op.add)
                    else:
                        gt = lpool.tile([L, ch], f32, tag="gt")
                        nc.sync.dma_start(out=gt[:], in_=gumbel_h[tok, lo:lo + ch])
                        eng = nc.vector if (add_mode == "dve" or c % 2 == 0) else nc.gpsimd
                        eng.tensor_tensor(out=lt[:], in0=lt[:], in1=gt[:], op=Op.add)
                    # chunk max + within-chunk argmax (first occurrence);
                    # the column copies go to the otherwise-idle ACT engine
                    mx8 = small.tile([L, 8], f32, tag="mx8")
                    nc.vector.max(out=mx8[:], in_=lt[:])
                    if smalls_on_act:
                        nc.scalar.copy(out=m_all[:, c:c + 1], in_=mx8[:, 0:1])
                    else:
                        nc.vector.tensor_copy(out=m_all[:, c:c + 1], in_=mx8[:, 0:1])
                    mi8 = small.tile([L, 8], u32, tag="mi8")
                    if not skip_maxidx:
                        nc.vector.max_index(out=mi8[:], in_max=mx8[:], in_values=lt[:])
                    else:
                        nc.vector.memset(mi8[:], 0)
                    if smalls_on_act:
                        nc.scalar.copy(out=idx_all[:, c:c + 1], in_=mi8[:, 0:1])
                    else:
                        nc.vector.tensor_copy(out=idx_all[:, c:c + 1], in_=mi8[:, 0:1])
                return m_all, idx_all

            def tail_phase(t, m_all, idx_all, mask_f, flag, psgemb):
                tok = slice(t * L, (t + 1) * L)
                # global max + first chunk attaining it
                gmax = small.tile([L, 1], f32, tag="gmax")
                nc.vector.reduce_max(out=gmax[:], in_=m_all[:], axis=AX.X)
                eq8 = small.tile([L, nch], f32, tag="eq8")
                nc.vector.tensor_scalar(eq8[:], m_all[:], gmax[:, 0:1], None, op0=Op.is_ge)
                sel8 = small.tile([L, nch], f32, tag="sel8")
                nc.vector.tensor_tensor(out=sel8[:], in0=eq8[:], in1=c8rev[:], op=Op.mult)
                cmax = small.tile([L, 1], f32, tag="cmax")
                nc.vector.reduce_max(out=cmax[:], in_=sel8[:], axis=AX.X)
                cstar = small.tile([L, 1], f32, tag="cstar")
                nc.vector.tensor_scalar(cstar[:], cmax[:], -1.0, float(nch), op0=Op.mult, op1=Op.add)
                # within-chunk index of the winning chunk: dot(onehot(cstar), idx_all)
                oh8 = small.tile([L, nch], f32, tag="oh8")
                nc.vector.tensor_scalar(oh8[:], iota8[:], cstar[:, 0:1], None, op0=Op.is_equal)
                junk8 = small.tile([L, nch], f32, tag="junk8")
                nc.vector.tensor_tensor(out=junk8[:], in0=oh8[:], in1=idx_all[:], op=Op.mult)
                mi_sel = small.tile([L, 1], f32, tag="mi_sel")
                nc.vector.reduce_max(out=mi_sel[:], in_=junk8[:], axis=AX.X)
                hot_f = small.tile([L, 1], f32, tag="hot_f")
                nc.vector.scalar_tensor_tensor(
                    out=hot_f[:], in0=cstar[:], scalar=float(ch), in1=mi_sel[:],
                    op0=Op.mult, op1=Op.add,
                )
                hot_i = small.tile([L, 1], i32, tag="hot_i")
                nc.vector.tensor_copy(out=hot_i[:], in_=hot_f[:])

                # hot -> vocab row (grid_sample LUT), -> token embeddings
                rowidx = small.tile([L, 1], i32, tag="rowidx")
                nc.gpsimd.indirect_dma_start(
                    out=rowidx[:], out_offset=None, in_=rowmap_h[:],
                    in_offset=IndirectOffsetOnAxis(ap=hot_i[:, 0:1], axis=0),
                )
                tokemb = emb.tile([L, e], f32, tag="tokemb")
                nc.gpsimd.indirect_dma_start(
                    out=tokemb[:], out_offset=None, in_=wtok_h[:],
                    in_offset=IndirectOffsetOnAxis(ap=rowidx[:, 0:1], axis=0),
                )

                # combine + store
                p1 = emb.tile([L, e], f32, tag="p1")
                nc.vector.tensor_scalar(p1[:], tokemb[:], mask_f[:, 0:1], None, op0=Op.mult)
                outt = emb.tile([L, e], f32, tag="outt")
                nc.vector.scalar_tensor_tensor(
                    out=outt[:], in0=psgemb[:], scalar=flag[:, 0:1], in1=p1[:],
                    op0=Op.mult, op1=Op.add)
                nc.sync.dma_start(out=out_h[tok, :], in_=outt[:])

            for _ in range(body_reps):
                if skip_tail:
                    for t in range(b_loc):
                        m_all, idx_all = stream_phase(t)
                        tok = slice(t * L, (t + 1) * L)
                        dummy = emb.tile([L, e], f32, tag="outt")
                        nc.vector.tensor_scalar(dummy[:], m_all[:, 0:1].to_broadcast([L, e]), 1.0, None, op0=Op.mult)
                        nc.sync.dma_start(out=out_h[tok, :], in_=dummy[:])
                    continue
                psg_state = [psg_phase(t) for t in range(b_loc)]
                if tail_after_each:
                    for t in range(b_loc):
                        m_all, idx_all = stream_phase(t)
                        tail_phase(t, m_all, idx_all, *psg_state[t])
                else:
                    streams = [stream_phase(t) for t in range(b_loc)]
                    for t in range(b_loc):
                        tail_phase(t, *streams[t], *psg_state[t])

    return nc


_BUILD_CACHE = {}


def _get_module(dims_key=None, dims=None, body_reps=1):
    key = (dims_key, body_reps)
    if key not in _BUILD_CACHE:
        import concourse.bacc as bacc

        nc = bacc.Bacc("TRN2", target_bir_lowering=False, debug=False)
        _build(nc, dims, body_reps=body_reps)
        nc.compile()
        _BUILD_CACHE[key] = nc
    return _BUILD_CACHE[key]


_ROWMAP_CACHE = {}


def _nearest_maps():
    """Replicate the reference's f32 grid_sample-nearest index maps with jnp
    on the same backend the reference runs on (bit-exact by construction)."""
    if "maps" not in _ROWMAP_CACHE:
        import jax.numpy as jnp

        def nearest(size):
            lin = jnp.linspace(-1.0, 1.0, size)
            ix = ((lin + 1.0) * size - 1.0) / 2.0
            return np.asarray(jnp.clip(jnp.round(ix), 0, size - 1).astype(jnp.int32))

        _ROWMAP_CACHE["maps"] = (nearest(V), nearest(E))
    return _ROWMAP_CACHE["maps"]


_TRI = None

# test/dev hooks: set TRACE=True before calling kernel() to capture an NTFF
# profile; the BassKernelResults of the last run is stored in LAST_RESULT.
TRACE = False
LAST_RESULT = None


def kernel(logits, rwrt_attention_mask, psg_input_ids, word_embeddings, gumbel_noise):
    from concourse.bass_utils import run_bass_kernel_spmd

    global _TRI
    logits = np.ascontiguousarray(np.asarray(logits, dtype=np.float32))
    gumbel = np.ascontiguousarray(np.asarray(gumbel_noise, dtype=np.float32))
    mask = np.ascontiguousarray(np.asarray(rwrt_attention_mask, dtype=np.int32))
    psg = np.ascontiguousarray(np.asarray(psg_input_ids, dtype=np.int32))
    wte = np.ascontiguousarray(np.asarray(word_embeddings, dtype=np.float32))

    rowmap, colmap = _nearest_maps()
    col_identity = bool(np.array_equal(colmap, np.arange(E, dtype=np.int32)))
    rowmap2 = rowmap.reshape(V, 1)
    if _TRI is None:
        _TRI = np.ascontiguousarray(np.triu(np.ones((L, L), dtype=np.float32)))

    if col_identity:
        nc = _get_module()
    else:
        # safety fallback (never taken in this environment): bake the column
        # permutation into a separate token-branch table
        nc = _get_module(dims_key="two_tables", dims={"TWO_TABLES": True})
        wte_tok = np.ascontiguousarray(wte[:, colmap])

    in_maps = []
    for m in range(N_CORES):
        sl = slice(m * B_LOC, (m + 1) * B_LOC)
        im = {
            "logits": logits[sl].reshape(B_LOC * L, V),
            "gumbel": gumbel[sl].reshape(B_LOC * L, V),
            "mask": mask[sl].reshape(B_LOC * L, 1),
            "psg": psg[sl].reshape(B_LOC * L, 1),
            "wte": wte,
            "rowmap": rowmap2,
            "tri": _TRI,
            "chain": np.zeros((L, 8), np.float32),
        }
        if not col_identity:
            im["wte_tok"] = wte_tok
        in_maps.append(im)

    global LAST_RESULT
    LAST_RESULT = run_bass_kernel_spmd(nc, in_maps, list(range(N_CORES)), trace=TRACE)
    res = LAST_RESULT.results
    out = np.concatenate(
        [res[m]["out"].reshape(B_LOC, L, E) for m in range(N_CORES)], axis=0
    )
    return out


# revision 34
# speedup vs baseline: 1.0368x; 1.0368x over previous
"""Trainium2 Bass kernel: gumbel-softmax-argmax embedding lookup (end-to-end).

Reference math (nn_End2End_49495203119139):
    hot  = argmax_V(softmax((logits + gumbel)/tau))       == argmax_V(logits+gumbel)
    row  = grid_sample-nearest index map of hot            == ROWMAP[hot]  (LUT)
    tok_emb = W[row][:, col_map]   with col_map == arange(E)  (verified at runtime)
    inputs_embeds = tok_emb * mask
    psg_roll = roll(psg_ids, 1, axis=1); psg_roll[:,0] = 1
    extr  = (1 - mask[:, ::-1]) * psg_roll
    trunc = rotate_right(extr, shifts) with shifts = mask.sum(-1)   (per row)
    flag  = cumsum(trunc != 0, -1) > 0
    out   = inputs_embeds + where(flag, W[trunc], 0)

Sharding: data-parallel over batch. B=16 over 8 cores -> 2 batch rows
(= 2 token tiles of 128) per core; the embedding table is replicated.

Per-core device plan (memory-bound part = streaming logits+gumbel, 66 MB,
~184 us HBM floor at ~358 GB/s per core):
  - for each token tile (128 tokens on partitions) and each vocab chunk
    [128 x 2008]: HWDGE-load the logits chunk, then add the gumbel chunk
    with one SWDGE inline-accumulate DMA (CCE add; descriptors must stay
    <= 2048 elements — larger accumulates crash the device).
    DVE `max` finds the chunk max, `max_index` the first within-chunk
    argmax position (ties resolve to the lowest index, matching argmax).
  - chunk winner (lowest chunk attaining the global max) + within-chunk
    index give `hot`; ROWMAP and W rows come via indirect DMA gathers.
  - the passage branch is pure index arithmetic on [128,1] tiles: the
    reverse/roll/rotate are folded into gather indices modulo L, the
    mask-sum and cumsum are 0/1 matmuls against ones/triangular matrices
    (exact in any PE precision).
Predicted 214.7 us/core by the TimelineSim cost model (DMA engines busy
190 us of that, i.e. ~97% of the 66MB/358GB/s floor); a hardware
min-slope measurement of the 4016-chunk variant gave ~204 us.
"""

import numpy as np

B = 16
L = 128
V = 32128
E = 768
N_CORES = 8
B_LOC = B // N_CORES          # batch rows per core
CH = 2008                     # vocab chunk (free dim) per streamed tile;
                              # <= 2048 so a gumbel chunk is ONE CCE-add DMA
NCH = V // CH                 # 16 chunks
NEG = -3.0e38


def _build(nc_mod, dims=None, body_reps=1):
    """Build the per-core Bass module. dims allows small smoke-test builds;
    body_reps>1 repeats the whole body (for slope-based benchmarking)."""
    import concourse.tile as tile
    from concourse import bass, mybir
    from concourse.bass import IndirectOffsetOnAxis

    d = dims or {}
    v = d.get("V", V)
    e = d.get("E", E)
    ch = d.get("CH", CH)
    nch = v // ch
    b_loc = d.get("B_LOC", B_LOC)
    rows = b_loc * L
    lbufs = d.get("LBUFS", 8)
    skip_tail = d.get("SKIP_TAIL", False)
    skip_accum = d.get("SKIP_ACCUM", False)
    skip_maxidx = d.get("SKIP_MAXIDX", False)
    tail_after_each = d.get("TAIL_AFTER_EACH", False)
    # how logits+gumbel are summed: "accum" = DMA CCE inline add (SWDGE),
    # "dve" = plain loads + DVE adds, "split" = adds alternate DVE/GpSimd
    add_mode = d.get("ADD_MODE", "accum")
    max_eng = d.get("MAX_ENG", "dve")
    smalls_on_act = d.get("SMALLS_ON_ACT", False)

    nc = nc_mod
    f32 = mybir.dt.float32
    i32 = mybir.dt.int32
    u32 = mybir.dt.uint32
    Op = mybir.AluOpType
    AX = mybir.AxisListType

    two_tables = d.get("TWO_TABLES", False)

    logits_h = nc.dram_tensor("logits", [rows, v], f32, kind="ExternalInput")
    gumbel_h = nc.dram_tensor("gumbel", [rows, v], f32, kind="ExternalInput")
    mask_h = nc.dram_tensor("mask", [rows, 1], i32, kind="ExternalInput")
    psg_h = nc.dram_tensor("psg", [rows, 1], i32, kind="ExternalInput")
    wte_h = nc.dram_tensor("wte", [v, e], f32, kind="ExternalInput")
    # the token branch reads W[:, col_map]; col_map is the identity here, so
    # both branches normally share one table (TWO_TABLES is a safety fallback)
    wtok_h = nc.dram_tensor("wte_tok", [v, e], f32, kind="ExternalInput") if two_tables else wte_h
    rowmap_h = nc.dram_tensor("rowmap", [v, 1], i32, kind="ExternalInput")
    tri_h = nc.dram_tensor("tri", [L, L], f32, kind="ExternalInput")
    out_h = nc.dram_tensor("out", [rows, e], f32, kind="ExternalOutput")
    # tiny passthrough pair so a benchmark can chain executions back-to-back
    chain_h = nc.dram_tensor("chain", [L, 8], f32, kind="ExternalInput")
    chain_o = nc.dram_tensor("chain_out", [L, 8], f32, kind="ExternalOutput")

    with tile.TileContext(nc) as tc:
        with (
            tc.tile_pool(name="lpool", bufs=lbufs) as lpool,
            tc.tile_pool(name="stats", bufs=2) as stats,
            tc.tile_pool(name="small", bufs=2) as small,
            tc.tile_pool(name="emb", bufs=2) as emb,
            tc.tile_pool(name="consts", bufs=1) as consts,
            tc.tile_pool(name="psum", bufs=2, space="PSUM") as psum,
        ):
            # ---- benchmark chain passthrough ----
            cht = consts.tile([L, 8], f32)
            nc.sync.dma_start(out=cht[:], in_=chain_h[:])
            nc.sync.dma_start(out=chain_o[:], in_=cht[:])

            # ---- per-core constants (built once) ----
            ones_mat = consts.tile([L, L], f32)
            nc.vector.memset(ones_mat[:], 1.0)
            tri_sb = consts.tile([L, L], f32)
            nc.sync.dma_start(out=tri_sb[:], in_=tri_h[:])

            iota_p_i = consts.tile([L, 1], i32)
            nc.gpsimd.iota(iota_p_i[:], pattern=[[1, 1]], base=0, channel_multiplier=1)
            iota_p = consts.tile([L, 1], f32)
            nc.vector.tensor_copy(out=iota_p[:], in_=iota_p_i[:])

            iota8_i = consts.tile([L, nch], i32)
            nc.gpsimd.iota(iota8_i[:], pattern=[[1, nch]], base=0, channel_multiplier=0)
            iota8 = consts.tile([L, nch], f32)
            nc.vector.tensor_copy(out=iota8[:], in_=iota8_i[:])
            # c8rev[c] = nch - c  (used to pick the LOWEST chunk that attains the max)
            c8rev = consts.tile([L, nch], f32)
            nc.vector.tensor_scalar(c8rev[:], iota8[:], -1.0, float(nch), op0=Op.mult, op1=Op.add)

            def psg_phase(t):
                """Everything that does not depend on the streamed logits:
                mask/psg index arithmetic, flag, psg-embedding gather."""
                tok = slice(t * L, (t + 1) * L)
                mask_i = small.tile([L, 1], i32, tag="mask_i")
                nc.sync.dma_start(out=mask_i[:], in_=mask_h[tok, :])
                mask_f = small.tile([L, 1], f32, tag="mask_f")
                nc.vector.tensor_copy(out=mask_f[:], in_=mask_i[:])

                # s (broadcast to all partitions) = sum(mask) via ones matmul
                s_ps = psum.tile([L, 1], f32, tag="s_ps")
                nc.tensor.matmul(out=s_ps[:], lhsT=ones_mat[:], rhs=mask_f[:], start=True, stop=True)
                s_bc = small.tile([L, 1], f32, tag="s_bc")
                nc.vector.tensor_copy(out=s_bc[:], in_=s_ps[:])

                def mod_l(x_ap, lo_fix=True, hi_fix=True, tagp=""):
                    # x <- x mod L for x in (-L, 2L)
                    if hi_fix:
                        ge = small.tile([L, 1], f32, tag="ge" + tagp)
                        nc.vector.tensor_scalar(ge[:], x_ap, float(L), None, op0=Op.is_ge)
                        nc.vector.scalar_tensor_tensor(
                            out=x_ap, in0=ge[:], scalar=-float(L), in1=x_ap, op0=Op.mult, op1=Op.add)
                    if lo_fix:
                        lt_ = small.tile([L, 1], f32, tag="lt" + tagp)
                        nc.vector.tensor_scalar(lt_[:], x_ap, 0.0, None, op0=Op.is_lt)
                        nc.vector.scalar_tensor_tensor(
                            out=x_ap, in0=lt_[:], scalar=float(L), in1=x_ap, op0=Op.mult, op1=Op.add)

                # fidx = (L-1 + s - l) mod L   (flipped-mask gather index)
                fidx = small.tile([L, 1], f32, tag="fidx")
                nc.vector.scalar_tensor_tensor(
                    out=fidx[:], in0=s_bc[:], scalar=float(L - 1), in1=iota_p[:],
                    op0=Op.add, op1=Op.subtract)
                mod_l(fidx[:], lo_fix=False, tagp="f")

                # pidx = (L-1 - s + l) mod L   (rolled-psg gather index)
                pidx = small.tile([L, 1], f32, tag="pidx")
                nc.vector.scalar_tensor_tensor(
                    out=pidx[:], in0=s_bc[:], scalar=-1.0, in1=iota_p[:],
                    op0=Op.mult, op1=Op.add)
                nc.vector.tensor_scalar(pidx[:], pidx[:], float(L - 1), None, op0=Op.add)
                mod_l(pidx[:], tagp="p")

                # k = (l - s) mod L ; BOS position is k == 0
                kk = small.tile([L, 1], f32, tag="kk")
                nc.vector.scalar_tensor_tensor(
                    out=kk[:], in0=s_bc[:], scalar=-1.0, in1=iota_p[:],
                    op0=Op.mult, op1=Op.add)
                mod_l(kk[:], hi_fix=False, tagp="k")
                bos = small.tile([L, 1], f32, tag="bos")
                nc.vector.tensor_scalar(bos[:], kk[:], 0.0, None, op0=Op.is_equal)

                # gather mask[fidx] and psg[pidx] (within this batch row)
                fr_i = small.tile([L, 1], i32, tag="fr_i")
                nc.vector.tensor_scalar(fidx[:], fidx[:], float(t * L), None, op0=Op.add)
                nc.vector.tensor_copy(out=fr_i[:], in_=fidx[:])
                mrev = small.tile([L, 1], i32, tag="mrev")
                nc.gpsimd.indirect_dma_start(
                    out=mrev[:], out_offset=None, in_=mask_h[:],
                    in_offset=IndirectOffsetOnAxis(ap=fr_i[:, 0:1], axis=0),
                )
                pr_i = small.tile([L, 1], i32, tag="pr_i")
                nc.vector.tensor_scalar(pidx[:], pidx[:], float(t * L), None, op0=Op.add)
                nc.vector.tensor_copy(out=pr_i[:], in_=pidx[:])
                prot = small.tile([L, 1], i32, tag="prot")
                nc.gpsimd.indirect_dma_start(
                    out=prot[:], out_offset=None, in_=psg_h[:],
                    in_offset=IndirectOffsetOnAxis(ap=pr_i[:, 0:1], axis=0),
                )

                # f_rot = 1 - mask[fidx]
                mrev_f = small.tile([L, 1], f32, tag="mrev_f")
                nc.vector.tensor_copy(out=mrev_f[:], in_=mrev[:])
                frot = small.tile([L, 1], f32, tag="frot")
                nc.vector.tensor_scalar(frot[:], mrev_f[:], -1.0, 1.0, op0=Op.mult, op1=Op.add)
                # psg_rot = bos ? 1 : psg[pidx]
                prot_f = small.tile([L, 1], f32, tag="prot_f")
                nc.vector.tensor_copy(out=prot_f[:], in_=prot[:])
                nbos = small.tile([L, 1], f32, tag="nbos")
                nc.vector.tensor_scalar(nbos[:], bos[:], -1.0, 1.0, op0=Op.mult, op1=Op.add)
                nc.vector.tensor_tensor(out=prot_f[:], in0=prot_f[:], in1=nbos[:], op=Op.mult)
                nc.vector.tensor_tensor(out=prot_f[:], in0=prot_f[:], in1=bos[:], op=Op.add)
                # trunc = f_rot * psg_rot
                trunc = small.tile([L, 1], f32, tag="trunc")
                nc.vector.tensor_tensor(out=trunc[:], in0=frot[:], in1=prot_f[:], op=Op.mult)

                # flag = cumsum(trunc != 0) > 0 via triangular matmul
                nz = small.tile([L, 1], f32, tag="nz")
                nc.vector.tensor_scalar(nz[:], trunc[:], 0.0, None, op0=Op.not_equal)
                cnt_ps = psum.tile([L, 1], f32, tag="cnt_ps")
                nc.tensor.matmul(out=cnt_ps[:], lhsT=tri_sb[:], rhs=nz[:], start=True, stop=True)
                flag = small.tile([L, 1], f32, tag="flag")
                nc.vector.tensor_scalar(flag[:], cnt_ps[:], 0.0, None, op0=Op.is_gt)

                trunc_i = small.tile([L, 1], i32, tag="trunc_i")
                nc.vector.tensor_copy(out=trunc_i[:], in_=trunc[:])
                psgemb = emb.tile([L, e], f32, tag="psgemb")
                nc.gpsimd.indirect_dma_start(
                    out=psgemb[:], out_offset=None, in_=wte_h[:],
                    in_offset=IndirectOffsetOnAxis(ap=trunc_i[:, 0:1], axis=0),
                )
                return mask_f, flag, psgemb

            def stream_phase(t):
                """DMA-bound pass over the vocab: per chunk, load logits,
                accumulate gumbel in the DMA datapath, track max + argmax."""
                tok = slice(t * L, (t + 1) * L)
                m_all = stats.tile([L, nch], f32, tag="m_all")
                idx_all = stats.tile([L, nch], f32, tag="idx_all")
                for c in range(nch):
                    lt = lpool.tile([L, ch], f32, tag="lt")
                    lo = c * ch
                    nc.sync.dma_start(out=lt[:], in_=logits_h[tok, lo:lo + ch])
                    if add_mode == "accum":
                        # s = logits + gumbel via DMA CCE inline add;
                        # descriptors must stay <= 2048 elements each.
                        half = ch // 2
                        if not skip_accum and ch <= 2048:
                            nc.gpsimd.dma_start(
                                out=lt[:], in_=gumbel_h[tok, lo:lo + ch],
                                accum_op=Op.add)
                        elif not skip_accum:
                            if d.get("ACCUM3D", False):
                                gsrc = gumbel_h[tok, lo:lo + ch].rearrange(
                                    "p (a b) -> p a b", b=half)
                                ldst = lt[:].rearrange("p (a b) -> p a b", b=half)
                                nc.gpsimd.dma_start(out=ldst, in_=gsrc, accum_op=Op.add)
                            else:
                                nc.gpsimd.dma_start(
                                    out=lt[:, 0:half], in_=gumbel_h[tok, lo:lo + half],
                                    accum_op=Op.add)
                                nc.gpsimd.dma_start(
                                    out=lt[:, half:ch], in_=gumbel_h[tok, lo + half:lo + ch],
                                    accum_op=Op.add)
                    elif add_mode == "hybrid":
                        # half the gumbel chunk via SWDGE inline-add DMA,
                        # half via HWDGE load + DVE add: balances Pool.SEQ
                        # descriptor emission against DVE cycles.
                        half = ch // 2
                        nc.gpsimd.dma_start(
                            out=lt[:, 0:half], in_=gumbel_h[tok, lo:lo + half],
                            accum_op=Op.add)
                        gt = lpool.tile([L, half], f32, tag="gt")
                        nc.sync.dma_start(out=gt[:], in_=gumbel_h[tok, lo + half:lo + ch])
                        nc.vector.tensor_tensor(out=lt[:, half:ch], in0=lt[:, half:ch], in1=gt[:], op=Op.add)
                    else:
                        gt = lpool.tile([L, ch], f32, tag="gt")
                        nc.sync.dma_start(out=gt[:], in_=gumbel_h[tok, lo:lo + ch])
                        eng = nc.vector if (add_mode == "dve" or c % 2 == 0) else nc.gpsimd
                        eng.tensor_tensor(out=lt[:], in0=lt[:], in1=gt[:], op=Op.add)
                    # chunk max + within-chunk argmax (first occurrence);
                    # the column copies go to the otherwise-idle ACT engine
                    mx8 = small.tile([L, 8], f32, tag="mx8")
                    nc.vector.max(out=mx8[:], in_=lt[:])
                    if smalls_on_act:
                        nc.scalar.copy(out=m_all[:, c:c + 1], in_=mx8[:, 0:1])
                    else:
                        nc.vector.tensor_copy(out=m_all[:, c:c + 1], in_=mx8[:, 0:1])
                    mi8 = small.tile([L, 8], u32, tag="mi8")
                    if not skip_maxidx:
                        nc.vector.max_index(out=mi8[:], in_max=mx8[:], in_values=lt[:])
                    else:
                        nc.vector.memset(mi8[:], 0)
                    if smalls_on_act:
                        nc.scalar.copy(out=idx_all[:, c:c + 1], in_=mi8[:, 0:1])
                    else:
                        nc.vector.tensor_copy(out=idx_all[:, c:c + 1], in_=mi8[:, 0:1])
                return m_all, idx_all

            def tail_phase(t, m_all, idx_all, mask_f, flag, psgemb):
                tok = slice(t * L, (t + 1) * L)
                # global max + first chunk attaining it
                gmax = small.tile([L, 1], f32, tag="gmax")
                nc.vector.reduce_max(out=gmax[:], in_=m_all[:], axis=AX.X)
                eq8 = small.tile([L, nch], f32, tag="eq8")
                nc.vector.tensor_scalar(eq8[:], m_all[:], gmax[:, 0:1], None, op0=Op.is_ge)
                sel8 = small.tile([L, nch], f32, tag="sel8")
                nc.vector.tensor_tensor(out=sel8[:], in0=eq8[:], in1=c8rev[:], op=Op.mult)
                cmax = small.tile([L, 1], f32, tag="cmax")
                nc.vector.reduce_max(out=cmax[:], in_=sel8[:], axis=AX.X)
                cstar = small.tile([L, 1], f32, tag="cstar")
                nc.vector.tensor_scalar(cstar[:], cmax[:], -1.0, float(nch), op0=Op.mult, op1=Op.add)
                # within-chunk index of the winning chunk: dot(onehot(cstar), idx_all)
                oh8 = small.tile([L, nch], f32, tag="oh8")
                nc.vector.tensor_scalar(oh8[:], iota8[:], cstar[:, 0:1], None, op0=Op.is_equal)
                junk8 = small.tile([L, nch], f32, tag="junk8")
                nc.vector.tensor_tensor(out=junk8[:], in0=oh8[:], in1=idx_all[:], op=Op.mult)
                mi_sel = small.tile([L, 1], f32, tag="mi_sel")
                nc.vector.reduce_max(out=mi_sel[:], in_=junk8[:], axis=AX.X)
                hot_f = small.tile([L, 1], f32, tag="hot_f")
                nc.vector.scalar_tensor_tensor(
                    out=hot_f[:], in0=cstar[:], scalar=float(ch), in1=mi_sel[:],
                    op0=Op.mult, op1=Op.add,
                )
                hot_i = small.tile([L, 1], i32, tag="hot_i")
                nc.vector.tensor_copy(out=hot_i[:], in_=hot_f[:])

                # hot -> vocab row (grid_sample LUT), -> token embeddings
                rowidx = small.tile([L, 1], i32, tag="rowidx")
                nc.gpsimd.indirect_dma_start(
                    out=rowidx[:], out_offset=None, in_=rowmap_h[:],
                    in_offset=IndirectOffsetOnAxis(ap=hot_i[:, 0:1], axis=0),
                )
                tokemb = emb.tile([L, e], f32, tag="tokemb")
                nc.gpsimd.indirect_dma_start(
                    out=tokemb[:], out_offset=None, in_=wtok_h[:],
                    in_offset=IndirectOffsetOnAxis(ap=rowidx[:, 0:1], axis=0),
                )

                # combine + store
                p1 = emb.tile([L, e], f32, tag="p1")
                nc.vector.tensor_scalar(p1[:], tokemb[:], mask_f[:, 0:1], None, op0=Op.mult)
                outt = emb.tile([L, e], f32, tag="outt")
                nc.vector.scalar_tensor_tensor(
                    out=outt[:], in0=psgemb[:], scalar=flag[:, 0:1], in1=p1[:],
                    op0=Op.mult, op1=Op.add)
                nc.sync.dma_start(out=out_h[tok, :], in_=outt[:])

            for _ in range(body_reps):
                if skip_tail:
                    for t in range(b_loc):
                        m_all, idx_all = stream_phase(t)
                        tok = slice(t * L, (t + 1) * L)
                        dummy = emb.tile([L, e], f32, tag="outt")
                        nc.vector.tensor_scalar(dummy[:], m_all[:, 0:1].to_broadcast([L, e]), 1.0, None, op0=Op.mult)
                        nc.sync.dma_start(out=out_h[tok, :], in_=dummy[:])
                    continue
                psg_state = [psg_phase(t) for t in range(b_loc)]
                if tail_after_each:
                    for t in range(b_loc):
                        m_all, idx_all = stream_phase(t)
                        tail_phase(t, m_all, idx_all, *psg_state[t])
                else:
                    streams = [stream_phase(t) for t in range(b_loc)]
                    for t in range(b_loc):
                        tail_phase(t, *streams[t], *psg_state[t])

    return nc


_BUILD_CACHE = {}


def _get_module(dims_key=None, dims=None, body_reps=1):
    key = (dims_key, body_reps)
    if key not in _BUILD_CACHE:
        import concourse.bacc as bacc

        nc = bacc.Bacc("TRN2", target_bir_lowering=False, debug=False)
        _build(nc, dims, body_reps=body_reps)
        nc.compile()
        _BUILD_CACHE[key] = nc
    return _BUILD_CACHE[key]


_ROWMAP_CACHE = {}


def _nearest_maps():
    """Replicate the reference's f32 grid_sample-nearest index maps with jnp
    on the same backend the reference runs on (bit-exact by construction)."""
    if "maps" not in _ROWMAP_CACHE:
        import jax.numpy as jnp

        def nearest(size):
            lin = jnp.linspace(-1.0, 1.0, size)
            ix = ((lin + 1.0) * size - 1.0) / 2.0
            return np.asarray(jnp.clip(jnp.round(ix), 0, size - 1).astype(jnp.int32))

        _ROWMAP_CACHE["maps"] = (nearest(V), nearest(E))
    return _ROWMAP_CACHE["maps"]


_TRI = None

# test/dev hooks: set TRACE=True before calling kernel() to capture an NTFF
# profile; the BassKernelResults of the last run is stored in LAST_RESULT.
TRACE = False
LAST_RESULT = None


def kernel(logits, rwrt_attention_mask, psg_input_ids, word_embeddings, gumbel_noise):
    from concourse.bass_utils import run_bass_kernel_spmd

    global _TRI
    logits = np.ascontiguousarray(np.asarray(logits, dtype=np.float32))
    gumbel = np.ascontiguousarray(np.asarray(gumbel_noise, dtype=np.float32))
    mask = np.ascontiguousarray(np.asarray(rwrt_attention_mask, dtype=np.int32))
    psg = np.ascontiguousarray(np.asarray(psg_input_ids, dtype=np.int32))
    wte = np.ascontiguousarray(np.asarray(word_embeddings, dtype=np.float32))

    rowmap, colmap = _nearest_maps()
    col_identity = bool(np.array_equal(colmap, np.arange(E, dtype=np.int32)))
    rowmap2 = rowmap.reshape(V, 1)
    if _TRI is None:
        _TRI = np.ascontiguousarray(np.triu(np.ones((L, L), dtype=np.float32)))

    if col_identity:
        nc = _get_module()
    else:
        # safety fallback (never taken in this environment): bake the column
        # permutation into a separate token-branch table
        nc = _get_module(dims_key="two_tables", dims={"TWO_TABLES": True})
        wte_tok = np.ascontiguousarray(wte[:, colmap])

    in_maps = []
    for m in range(N_CORES):
        sl = slice(m * B_LOC, (m + 1) * B_LOC)
        im = {
            "logits": logits[sl].reshape(B_LOC * L, V),
            "gumbel": gumbel[sl].reshape(B_LOC * L, V),
            "mask": mask[sl].reshape(B_LOC * L, 1),
            "psg": psg[sl].reshape(B_LOC * L, 1),
            "wte": wte,
            "rowmap": rowmap2,
            "tri": _TRI,
            "chain": np.zeros((L, 8), np.float32),
        }
        if not col_identity:
            im["wte_tok"] = wte_tok
        in_maps.append(im)

    global LAST_RESULT
    LAST_RESULT = run_bass_kernel_spmd(nc, in_maps, list(range(N_CORES)), trace=TRACE)
    res = LAST_RESULT.results
    out = np.concatenate(
        [res[m]["out"].reshape(B_LOC, L, E) for m in range(N_CORES)], axis=0
    )
    return out


# revision 37
# speedup vs baseline: 1.0442x; 1.0072x over previous
"""Trainium2 Bass kernel: gumbel-softmax-argmax embedding lookup (end-to-end).

Reference math (nn_End2End_49495203119139):
    hot  = argmax_V(softmax((logits + gumbel)/tau))       == argmax_V(logits+gumbel)
    row  = grid_sample-nearest index map of hot            == ROWMAP[hot]  (LUT)
    tok_emb = W[row][:, col_map]   with col_map == arange(E)  (verified at runtime)
    inputs_embeds = tok_emb * mask
    psg_roll = roll(psg_ids, 1, axis=1); psg_roll[:,0] = 1
    extr  = (1 - mask[:, ::-1]) * psg_roll
    trunc = rotate_right(extr, shifts) with shifts = mask.sum(-1)   (per row)
    flag  = cumsum(trunc != 0, -1) > 0
    out   = inputs_embeds + where(flag, W[trunc], 0)

Sharding: data-parallel over batch. B=16 over 8 cores -> 2 batch rows
(= 2 token tiles of 128) per core; the embedding table is replicated.

Per-core device plan (memory-bound part = streaming logits+gumbel, 66 MB,
~184 us HBM floor at ~358 GB/s per core):
  - for each token tile (128 tokens on partitions) and each vocab chunk
    [128 x 2008]: HWDGE-load the logits chunk, then add the gumbel chunk
    with one SWDGE inline-accumulate DMA (CCE add; descriptors must stay
    <= 2048 elements — larger accumulates crash the device).
    DVE `max` finds the chunk max, `max_index` the first within-chunk
    argmax position (ties resolve to the lowest index, matching argmax).
  - chunk winner (lowest chunk attaining the global max) + within-chunk
    index give `hot`; ROWMAP and W rows come via indirect DMA gathers.
  - the passage branch is pure index arithmetic on [128,1] tiles: the
    reverse/roll/rotate are folded into gather indices modulo L, the
    mask-sum and cumsum are 0/1 matmuls against ones/triangular matrices
    (exact in any PE precision).
Predicted 213.2 us/core by the TimelineSim cost model (DMA engines busy
190 us of that, i.e. ~97% of the 66MB/358GB/s floor); a hardware
min-slope measurement of the 4016-chunk variant gave ~204 us.
"""

import numpy as np

B = 16
L = 128
V = 32128
E = 768
N_CORES = 8
B_LOC = B // N_CORES          # batch rows per core
CH = 2008                     # vocab chunk (free dim) per streamed tile;
                              # <= 2048 so a gumbel chunk is ONE CCE-add DMA
NCH = V // CH                 # 16 chunks
NEG = -3.0e38


def _build(nc_mod, dims=None, body_reps=1):
    """Build the per-core Bass module. dims allows small smoke-test builds;
    body_reps>1 repeats the whole body (for slope-based benchmarking)."""
    import concourse.tile as tile
    from concourse import bass, mybir
    from concourse.bass import IndirectOffsetOnAxis

    d = dims or {}
    v = d.get("V", V)
    e = d.get("E", E)
    ch = d.get("CH", CH)
    nch = v // ch
    b_loc = d.get("B_LOC", B_LOC)
    rows = b_loc * L
    lbufs = d.get("LBUFS", 8)
    skip_tail = d.get("SKIP_TAIL", False)
    skip_accum = d.get("SKIP_ACCUM", False)
    skip_maxidx = d.get("SKIP_MAXIDX", False)
    tail_after_each = d.get("TAIL_AFTER_EACH", False)
    # how logits+gumbel are summed: "accum" = DMA CCE inline add (SWDGE),
    # "dve" = plain loads + DVE adds, "split" = adds alternate DVE/GpSimd
    add_mode = d.get("ADD_MODE", "accum")
    max_eng = d.get("MAX_ENG", "dve")
    smalls_on_act = d.get("SMALLS_ON_ACT", False)

    nc = nc_mod
    f32 = mybir.dt.float32
    i32 = mybir.dt.int32
    u32 = mybir.dt.uint32
    Op = mybir.AluOpType
    AX = mybir.AxisListType

    two_tables = d.get("TWO_TABLES", False)

    logits_h = nc.dram_tensor("logits", [rows, v], f32, kind="ExternalInput")
    gumbel_h = nc.dram_tensor("gumbel", [rows, v], f32, kind="ExternalInput")
    mask_h = nc.dram_tensor("mask", [rows, 1], i32, kind="ExternalInput")
    psg_h = nc.dram_tensor("psg", [rows, 1], i32, kind="ExternalInput")
    wte_h = nc.dram_tensor("wte", [v, e], f32, kind="ExternalInput")
    # the token branch reads W[:, col_map]; col_map is the identity here, so
    # both branches normally share one table (TWO_TABLES is a safety fallback)
    wtok_h = nc.dram_tensor("wte_tok", [v, e], f32, kind="ExternalInput") if two_tables else wte_h
    rowmap_h = nc.dram_tensor("rowmap", [v, 1], i32, kind="ExternalInput")
    tri_h = nc.dram_tensor("tri", [L, L], f32, kind="ExternalInput")
    out_h = nc.dram_tensor("out", [rows, e], f32, kind="ExternalOutput")
    # tiny passthrough pair so a benchmark can chain executions back-to-back
    chain_h = nc.dram_tensor("chain", [L, 8], f32, kind="ExternalInput")
    chain_o = nc.dram_tensor("chain_out", [L, 8], f32, kind="ExternalOutput")

    with tile.TileContext(nc) as tc:
        with (
            tc.tile_pool(name="lpool", bufs=lbufs) as lpool,
            tc.tile_pool(name="stats", bufs=2) as stats,
            tc.tile_pool(name="small", bufs=2) as small,
            tc.tile_pool(name="emb", bufs=2) as emb,
            tc.tile_pool(name="consts", bufs=1) as consts,
            tc.tile_pool(name="psum", bufs=2, space="PSUM") as psum,
        ):
            # ---- benchmark chain passthrough ----
            cht = consts.tile([L, 8], f32)
            nc.sync.dma_start(out=cht[:], in_=chain_h[:])
            nc.sync.dma_start(out=chain_o[:], in_=cht[:])

            # ---- per-core constants (built once) ----
            ones_mat = consts.tile([L, L], f32)
            nc.vector.memset(ones_mat[:], 1.0)
            tri_sb = consts.tile([L, L], f32)
            nc.sync.dma_start(out=tri_sb[:], in_=tri_h[:])

            iota_p_i = consts.tile([L, 1], i32)
            nc.gpsimd.iota(iota_p_i[:], pattern=[[1, 1]], base=0, channel_multiplier=1)
            iota_p = consts.tile([L, 1], f32)
            nc.vector.tensor_copy(out=iota_p[:], in_=iota_p_i[:])

            iota8_i = consts.tile([L, nch], i32)
            nc.gpsimd.iota(iota8_i[:], pattern=[[1, nch]], base=0, channel_multiplier=0)
            iota8 = consts.tile([L, nch], f32)
            nc.vector.tensor_copy(out=iota8[:], in_=iota8_i[:])
            # c8rev[c] = nch - c  (used to pick the LOWEST chunk that attains the max)
            c8rev = consts.tile([L, nch], f32)
            nc.vector.tensor_scalar(c8rev[:], iota8[:], -1.0, float(nch), op0=Op.mult, op1=Op.add)

            def psg_phase(t):
                """Everything that does not depend on the streamed logits:
                mask/psg index arithmetic, flag, psg-embedding gather."""
                tok = slice(t * L, (t + 1) * L)
                mask_i = small.tile([L, 1], i32, tag="mask_i")
                nc.sync.dma_start(out=mask_i[:], in_=mask_h[tok, :])
                mask_f = small.tile([L, 1], f32, tag="mask_f")
                nc.vector.tensor_copy(out=mask_f[:], in_=mask_i[:])

                # s (broadcast to all partitions) = sum(mask) via ones matmul
                s_ps = psum.tile([L, 1], f32, tag="s_ps")
                nc.tensor.matmul(out=s_ps[:], lhsT=ones_mat[:], rhs=mask_f[:], start=True, stop=True)
                s_bc = small.tile([L, 1], f32, tag="s_bc")
                nc.vector.tensor_copy(out=s_bc[:], in_=s_ps[:])

                def mod_l(x_ap, lo_fix=True, hi_fix=True, tagp=""):
                    # x <- x mod L for x in (-L, 2L)
                    if hi_fix:
                        ge = small.tile([L, 1], f32, tag="ge" + tagp)
                        nc.vector.tensor_scalar(ge[:], x_ap, float(L), None, op0=Op.is_ge)
                        nc.vector.scalar_tensor_tensor(
                            out=x_ap, in0=ge[:], scalar=-float(L), in1=x_ap, op0=Op.mult, op1=Op.add)
                    if lo_fix:
                        lt_ = small.tile([L, 1], f32, tag="lt" + tagp)
                        nc.vector.tensor_scalar(lt_[:], x_ap, 0.0, None, op0=Op.is_lt)
                        nc.vector.scalar_tensor_tensor(
                            out=x_ap, in0=lt_[:], scalar=float(L), in1=x_ap, op0=Op.mult, op1=Op.add)

                # fidx = (L-1 + s - l) mod L   (flipped-mask gather index)
                fidx = small.tile([L, 1], f32, tag="fidx")
                nc.vector.scalar_tensor_tensor(
                    out=fidx[:], in0=s_bc[:], scalar=float(L - 1), in1=iota_p[:],
                    op0=Op.add, op1=Op.subtract)
                mod_l(fidx[:], lo_fix=False, tagp="f")

                # pidx = (L-1 - s + l) mod L   (rolled-psg gather index)
                pidx = small.tile([L, 1], f32, tag="pidx")
                nc.vector.scalar_tensor_tensor(
                    out=pidx[:], in0=s_bc[:], scalar=-1.0, in1=iota_p[:],
                    op0=Op.mult, op1=Op.add)
                nc.vector.tensor_scalar(pidx[:], pidx[:], float(L - 1), None, op0=Op.add)
                mod_l(pidx[:], tagp="p")

                # k = (l - s) mod L ; BOS position is k == 0
                kk = small.tile([L, 1], f32, tag="kk")
                nc.vector.scalar_tensor_tensor(
                    out=kk[:], in0=s_bc[:], scalar=-1.0, in1=iota_p[:],
                    op0=Op.mult, op1=Op.add)
                mod_l(kk[:], hi_fix=False, tagp="k")
                bos = small.tile([L, 1], f32, tag="bos")
                nc.vector.tensor_scalar(bos[:], kk[:], 0.0, None, op0=Op.is_equal)

                # gather mask[fidx] and psg[pidx] (within this batch row)
                fr_i = small.tile([L, 1], i32, tag="fr_i")
                nc.vector.tensor_scalar(fidx[:], fidx[:], float(t * L), None, op0=Op.add)
                nc.vector.tensor_copy(out=fr_i[:], in_=fidx[:])
                mrev = small.tile([L, 1], i32, tag="mrev")
                nc.gpsimd.indirect_dma_start(
                    out=mrev[:], out_offset=None, in_=mask_h[:],
                    in_offset=IndirectOffsetOnAxis(ap=fr_i[:, 0:1], axis=0),
                )
                pr_i = small.tile([L, 1], i32, tag="pr_i")
                nc.vector.tensor_scalar(pidx[:], pidx[:], float(t * L), None, op0=Op.add)
                nc.vector.tensor_copy(out=pr_i[:], in_=pidx[:])
                prot = small.tile([L, 1], i32, tag="prot")
                nc.gpsimd.indirect_dma_start(
                    out=prot[:], out_offset=None, in_=psg_h[:],
                    in_offset=IndirectOffsetOnAxis(ap=pr_i[:, 0:1], axis=0),
                )

                # f_rot = 1 - mask[fidx]
                mrev_f = small.tile([L, 1], f32, tag="mrev_f")
                nc.vector.tensor_copy(out=mrev_f[:], in_=mrev[:])
                frot = small.tile([L, 1], f32, tag="frot")
                nc.vector.tensor_scalar(frot[:], mrev_f[:], -1.0, 1.0, op0=Op.mult, op1=Op.add)
                # psg_rot = bos ? 1 : psg[pidx]
                prot_f = small.tile([L, 1], f32, tag="prot_f")
                nc.vector.tensor_copy(out=prot_f[:], in_=prot[:])
                nbos = small.tile([L, 1], f32, tag="nbos")
                nc.vector.tensor_scalar(nbos[:], bos[:], -1.0, 1.0, op0=Op.mult, op1=Op.add)
                nc.vector.tensor_tensor(out=prot_f[:], in0=prot_f[:], in1=nbos[:], op=Op.mult)
                nc.vector.tensor_tensor(out=prot_f[:], in0=prot_f[:], in1=bos[:], op=Op.add)
                # trunc = f_rot * psg_rot
                trunc = small.tile([L, 1], f32, tag="trunc")
                nc.vector.tensor_tensor(out=trunc[:], in0=frot[:], in1=prot_f[:], op=Op.mult)

                # flag = cumsum(trunc != 0) > 0 via triangular matmul
                nz = small.tile([L, 1], f32, tag="nz")
                nc.vector.tensor_scalar(nz[:], trunc[:], 0.0, None, op0=Op.not_equal)
                cnt_ps = psum.tile([L, 1], f32, tag="cnt_ps")
                nc.tensor.matmul(out=cnt_ps[:], lhsT=tri_sb[:], rhs=nz[:], start=True, stop=True)
                flag = small.tile([L, 1], f32, tag="flag")
                nc.vector.tensor_scalar(flag[:], cnt_ps[:], 0.0, None, op0=Op.is_gt)

                trunc_i = small.tile([L, 1], i32, tag="trunc_i")
                nc.vector.tensor_copy(out=trunc_i[:], in_=trunc[:])
                psgemb = emb.tile([L, e], f32, tag="psgemb")
                nc.gpsimd.indirect_dma_start(
                    out=psgemb[:], out_offset=None, in_=wte_h[:],
                    in_offset=IndirectOffsetOnAxis(ap=trunc_i[:, 0:1], axis=0),
                )
                return mask_f, flag, psgemb

            def stream_phase(t):
                """DMA-bound pass over the vocab: per chunk, load logits,
                accumulate gumbel in the DMA datapath, track max + argmax."""
                tok = slice(t * L, (t + 1) * L)
                m_all = stats.tile([L, nch], f32, tag="m_all")
                idx_all = stats.tile([L, nch], f32, tag="idx_all")
                for c in range(nch):
                    lt = lpool.tile([L, ch], f32, tag="lt")
                    lo = c * ch
                    ldeng = nc.scalar if (d.get("DUAL_HWDGE", True) and c % 2) else nc.sync
                    ldeng.dma_start(out=lt[:], in_=logits_h[tok, lo:lo + ch])
                    if add_mode == "accum":
                        # s = logits + gumbel via DMA CCE inline add;
                        # descriptors must stay <= 2048 elements each.
                        half = ch // 2
                        if not skip_accum and ch <= 2048:
                            nc.gpsimd.dma_start(
                                out=lt[:], in_=gumbel_h[tok, lo:lo + ch],
                                accum_op=Op.add)
                        elif not skip_accum:
                            if d.get("ACCUM3D", False):
                                gsrc = gumbel_h[tok, lo:lo + ch].rearrange(
                                    "p (a b) -> p a b", b=half)
                                ldst = lt[:].rearrange("p (a b) -> p a b", b=half)
                                nc.gpsimd.dma_start(out=ldst, in_=gsrc, accum_op=Op.add)
                            else:
                                nc.gpsimd.dma_start(
                                    out=lt[:, 0:half], in_=gumbel_h[tok, lo:lo + half],
                                    accum_op=Op.add)
                                nc.gpsimd.dma_start(
                                    out=lt[:, half:ch], in_=gumbel_h[tok, lo + half:lo + ch],
                                    accum_op=Op.add)
                    elif add_mode == "hybrid":
                        # half the gumbel chunk via SWDGE inline-add DMA,
                        # half via HWDGE load + DVE add: balances Pool.SEQ
                        # descriptor emission against DVE cycles.
                        half = ch // 2
                        nc.gpsimd.dma_start(
                            out=lt[:, 0:half], in_=gumbel_h[tok, lo:lo + half],
                            accum_op=Op.add)
                        gt = lpool.tile([L, half], f32, tag="gt")
                        nc.sync.dma_start(out=gt[:], in_=gumbel_h[tok, lo + half:lo + ch])
                        nc.vector.tensor_tensor(out=lt[:, half:ch], in0=lt[:, half:ch], in1=gt[:], op=Op.add)
                    else:
                        gt = lpool.tile([L, ch], f32, tag="gt")
                        nc.sync.dma_start(out=gt[:], in_=gumbel_h[tok, lo:lo + ch])
                        eng = nc.vector if (add_mode == "dve" or c % 2 == 0) else nc.gpsimd
                        eng.tensor_tensor(out=lt[:], in0=lt[:], in1=gt[:], op=Op.add)
                    # chunk max + within-chunk argmax (first occurrence);
                    # the column copies go to the otherwise-idle ACT engine
                    mx8 = small.tile([L, 8], f32, tag="mx8")
                    nc.vector.max(out=mx8[:], in_=lt[:])
                    if smalls_on_act:
                        nc.scalar.copy(out=m_all[:, c:c + 1], in_=mx8[:, 0:1])
                    else:
                        nc.vector.tensor_copy(out=m_all[:, c:c + 1], in_=mx8[:, 0:1])
                    mi8 = small.tile([L, 8], u32, tag="mi8")
                    if not skip_maxidx:
                        nc.vector.max_index(out=mi8[:], in_max=mx8[:], in_values=lt[:])
                    else:
                        nc.vector.memset(mi8[:], 0)
                    if smalls_on_act:
                        nc.scalar.copy(out=idx_all[:, c:c + 1], in_=mi8[:, 0:1])
                    else:
                        nc.vector.tensor_copy(out=idx_all[:, c:c + 1], in_=mi8[:, 0:1])
                return m_all, idx_all

            def tail_phase(t, m_all, idx_all, mask_f, flag, psgemb):
                tok = slice(t * L, (t + 1) * L)
                # global max + first chunk attaining it
                gmax = small.tile([L, 1], f32, tag="gmax")
                nc.vector.reduce_max(out=gmax[:], in_=m_all[:], axis=AX.X)
                eq8 = small.tile([L, nch], f32, tag="eq8")
                nc.vector.tensor_scalar(eq8[:], m_all[:], gmax[:, 0:1], None, op0=Op.is_ge)
                sel8 = small.tile([L, nch], f32, tag="sel8")
                nc.vector.tensor_tensor(out=sel8[:], in0=eq8[:], in1=c8rev[:], op=Op.mult)
                cmax = small.tile([L, 1], f32, tag="cmax")
                nc.vector.reduce_max(out=cmax[:], in_=sel8[:], axis=AX.X)
                cstar = small.tile([L, 1], f32, tag="cstar")
                nc.vector.tensor_scalar(cstar[:], cmax[:], -1.0, float(nch), op0=Op.mult, op1=Op.add)
                # within-chunk index of the winning chunk: dot(onehot(cstar), idx_all)
                oh8 = small.tile([L, nch], f32, tag="oh8")
                nc.vector.tensor_scalar(oh8[:], iota8[:], cstar[:, 0:1], None, op0=Op.is_equal)
                junk8 = small.tile([L, nch], f32, tag="junk8")
                nc.vector.tensor_tensor(out=junk8[:], in0=oh8[:], in1=idx_all[:], op=Op.mult)
                mi_sel = small.tile([L, 1], f32, tag="mi_sel")
                nc.vector.reduce_max(out=mi_sel[:], in_=junk8[:], axis=AX.X)
                hot_f = small.tile([L, 1], f32, tag="hot_f")
                nc.vector.scalar_tensor_tensor(
                    out=hot_f[:], in0=cstar[:], scalar=float(ch), in1=mi_sel[:],
                    op0=Op.mult, op1=Op.add,
                )
                hot_i = small.tile([L, 1], i32, tag="hot_i")
                nc.vector.tensor_copy(out=hot_i[:], in_=hot_f[:])

                # hot -> vocab row (grid_sample LUT), -> token embeddings
                rowidx = small.tile([L, 1], i32, tag="rowidx")
                nc.gpsimd.indirect_dma_start(
                    out=rowidx[:], out_offset=None, in_=rowmap_h[:],
                    in_offset=IndirectOffsetOnAxis(ap=hot_i[:, 0:1], axis=0),
                )
                tokemb = emb.tile([L, e], f32, tag="tokemb")
                nc.gpsimd.indirect_dma_start(
                    out=tokemb[:], out_offset=None, in_=wtok_h[:],
                    in_offset=IndirectOffsetOnAxis(ap=rowidx[:, 0:1], axis=0),
                )

                # combine + store
                p1 = emb.tile([L, e], f32, tag="p1")
                nc.vector.tensor_scalar(p1[:], tokemb[:], mask_f[:, 0:1], None, op0=Op.mult)
                outt = emb.tile([L, e], f32, tag="outt")
                nc.vector.scalar_tensor_tensor(
                    out=outt[:], in0=psgemb[:], scalar=flag[:, 0:1], in1=p1[:],
                    op0=Op.mult, op1=Op.add)
                nc.sync.dma_start(out=out_h[tok, :], in_=outt[:])

            for _ in range(body_reps):
                if skip_tail:
                    for t in range(b_loc):
                        m_all, idx_all = stream_phase(t)
                        tok = slice(t * L, (t + 1) * L)
                        dummy = emb.tile([L, e], f32, tag="outt")
                        nc.vector.tensor_scalar(dummy[:], m_all[:, 0:1].to_broadcast([L, e]), 1.0, None, op0=Op.mult)
                        nc.sync.dma_start(out=out_h[tok, :], in_=dummy[:])
                    continue
                psg_state = [psg_phase(t) for t in range(b_loc)]
                if tail_after_each:
                    for t in range(b_loc):
                        m_all, idx_all = stream_phase(t)
                        tail_phase(t, m_all, idx_all, *psg_state[t])
                else:
                    streams = [stream_phase(t) for t in range(b_loc)]
                    for t in range(b_loc):
                        tail_phase(t, *streams[t], *psg_state[t])

    return nc


_BUILD_CACHE = {}


def _get_module(dims_key=None, dims=None, body_reps=1):
    key = (dims_key, body_reps)
    if key not in _BUILD_CACHE:
        import concourse.bacc as bacc

        nc = bacc.Bacc("TRN2", target_bir_lowering=False, debug=False)
        _build(nc, dims, body_reps=body_reps)
        nc.compile()
        _BUILD_CACHE[key] = nc
    return _BUILD_CACHE[key]


_ROWMAP_CACHE = {}


def _nearest_maps():
    """Replicate the reference's f32 grid_sample-nearest index maps with jnp
    on the same backend the reference runs on (bit-exact by construction)."""
    if "maps" not in _ROWMAP_CACHE:
        import jax.numpy as jnp

        def nearest(size):
            lin = jnp.linspace(-1.0, 1.0, size)
            ix = ((lin + 1.0) * size - 1.0) / 2.0
            return np.asarray(jnp.clip(jnp.round(ix), 0, size - 1).astype(jnp.int32))

        _ROWMAP_CACHE["maps"] = (nearest(V), nearest(E))
    return _ROWMAP_CACHE["maps"]


_TRI = None

# test/dev hooks: set TRACE=True before calling kernel() to capture an NTFF
# profile; the BassKernelResults of the last run is stored in LAST_RESULT.
TRACE = False
LAST_RESULT = None


def kernel(logits, rwrt_attention_mask, psg_input_ids, word_embeddings, gumbel_noise):
    from concourse.bass_utils import run_bass_kernel_spmd

    global _TRI
    logits = np.ascontiguousarray(np.asarray(logits, dtype=np.float32))
    gumbel = np.ascontiguousarray(np.asarray(gumbel_noise, dtype=np.float32))
    mask = np.ascontiguousarray(np.asarray(rwrt_attention_mask, dtype=np.int32))
    psg = np.ascontiguousarray(np.asarray(psg_input_ids, dtype=np.int32))
    wte = np.ascontiguousarray(np.asarray(word_embeddings, dtype=np.float32))

    rowmap, colmap = _nearest_maps()
    col_identity = bool(np.array_equal(colmap, np.arange(E, dtype=np.int32)))
    rowmap2 = rowmap.reshape(V, 1)
    if _TRI is None:
        _TRI = np.ascontiguousarray(np.triu(np.ones((L, L), dtype=np.float32)))

    if col_identity:
        nc = _get_module()
    else:
        # safety fallback (never taken in this environment): bake the column
        # permutation into a separate token-branch table
        nc = _get_module(dims_key="two_tables", dims={"TWO_TABLES": True})
        wte_tok = np.ascontiguousarray(wte[:, colmap])

    in_maps = []
    for m in range(N_CORES):
        sl = slice(m * B_LOC, (m + 1) * B_LOC)
        im = {
            "logits": logits[sl].reshape(B_LOC * L, V),
            "gumbel": gumbel[sl].reshape(B_LOC * L, V),
            "mask": mask[sl].reshape(B_LOC * L, 1),
            "psg": psg[sl].reshape(B_LOC * L, 1),
            "wte": wte,
            "rowmap": rowmap2,
            "tri": _TRI,
            "chain": np.zeros((L, 8), np.float32),
        }
        if not col_identity:
            im["wte_tok"] = wte_tok
        in_maps.append(im)

    global LAST_RESULT
    try:
        LAST_RESULT = run_bass_kernel_spmd(nc, in_maps, list(range(N_CORES)), trace=TRACE)
    except Exception:
        # the axon-relayed device occasionally reports a transient
        # NRT_EXEC_UNIT_UNRECOVERABLE on the first execution after long
        # sessions; a straight re-run recovers it
        import time as _time

        _time.sleep(2.0)
        LAST_RESULT = run_bass_kernel_spmd(nc, in_maps, list(range(N_CORES)), trace=TRACE)
    res = LAST_RESULT.results
    out = np.concatenate(
        [res[m]["out"].reshape(B_LOC, L, E) for m in range(N_CORES)], axis=0
    )
    return out


# revision 38
# speedup vs baseline: 1.0448x; 1.0005x over previous
"""Trainium2 Bass kernel: gumbel-softmax-argmax embedding lookup (end-to-end).

Reference math (nn_End2End_49495203119139):
    hot  = argmax_V(softmax((logits + gumbel)/tau))       == argmax_V(logits+gumbel)
    row  = grid_sample-nearest index map of hot            == ROWMAP[hot]  (LUT)
    tok_emb = W[row][:, col_map]   with col_map == arange(E)  (verified at runtime)
    inputs_embeds = tok_emb * mask
    psg_roll = roll(psg_ids, 1, axis=1); psg_roll[:,0] = 1
    extr  = (1 - mask[:, ::-1]) * psg_roll
    trunc = rotate_right(extr, shifts) with shifts = mask.sum(-1)   (per row)
    flag  = cumsum(trunc != 0, -1) > 0
    out   = inputs_embeds + where(flag, W[trunc], 0)

Sharding: data-parallel over batch. B=16 over 8 cores -> 2 batch rows
(= 2 token tiles of 128) per core; the embedding table is replicated.

Per-core device plan (memory-bound part = streaming logits+gumbel, 66 MB,
~184 us HBM floor at ~358 GB/s per core):
  - for each token tile (128 tokens on partitions) and each vocab chunk
    [128 x 2008]: HWDGE-load the logits chunk, then add the gumbel chunk
    with one SWDGE inline-accumulate DMA (CCE add; descriptors must stay
    <= 2048 elements — larger accumulates crash the device).
    DVE `max` finds the chunk max, `max_index` the first within-chunk
    argmax position (ties resolve to the lowest index, matching argmax).
  - chunk winner (lowest chunk attaining the global max) + within-chunk
    index give `hot`; ROWMAP and W rows come via indirect DMA gathers.
  - the passage branch is pure index arithmetic on [128,1] tiles: the
    reverse/roll/rotate are folded into gather indices modulo L, the
    mask-sum and cumsum are 0/1 matmuls against ones/triangular matrices
    (exact in any PE precision).
Predicted 213.2 us/core by the TimelineSim cost model (DMA engines busy
190 us of that, i.e. ~97% of the 66MB/358GB/s floor); a hardware
min-slope measurement of the 4016-chunk variant gave ~204 us.
"""

import numpy as np

B = 16
L = 128
V = 32128
E = 768
N_CORES = 8
B_LOC = B // N_CORES          # batch rows per core
CH = 2008                     # vocab chunk (free dim) per streamed tile;
                              # <= 2048 so a gumbel chunk is ONE CCE-add DMA
NCH = V // CH                 # 16 chunks
NEG = -3.0e38


def _build(nc_mod, dims=None, body_reps=1):
    """Build the per-core Bass module. dims allows small smoke-test builds;
    body_reps>1 repeats the whole body (for slope-based benchmarking)."""
    import concourse.tile as tile
    from concourse import bass, mybir
    from concourse.bass import IndirectOffsetOnAxis

    d = dims or {}
    v = d.get("V", V)
    e = d.get("E", E)
    ch = d.get("CH", CH)
    nch = v // ch
    b_loc = d.get("B_LOC", B_LOC)
    rows = b_loc * L
    lbufs = d.get("LBUFS", 8)
    skip_tail = d.get("SKIP_TAIL", False)
    skip_accum = d.get("SKIP_ACCUM", False)
    skip_maxidx = d.get("SKIP_MAXIDX", False)
    tail_after_each = d.get("TAIL_AFTER_EACH", False)
    # how logits+gumbel are summed: "accum" = DMA CCE inline add (SWDGE),
    # "dve" = plain loads + DVE adds, "split" = adds alternate DVE/GpSimd
    add_mode = d.get("ADD_MODE", "accum")
    # chunk spans (lo, size); SMALL_LAST splits the final chunk so the
    # post-last-DMA DVE chain (max+max_index of the last chunk) is short
    spans = [(c * ch, ch) for c in range(nch)]
    if d.get("SMALL_LAST", False) and ch >= 1024:
        lo_last, sz = spans.pop()
        spans.append((lo_last, sz - 502))
        spans.append((lo_last + sz - 502, 502))
    nsp = len(spans)
    max_eng = d.get("MAX_ENG", "dve")
    smalls_on_act = d.get("SMALLS_ON_ACT", False)

    nc = nc_mod
    f32 = mybir.dt.float32
    i32 = mybir.dt.int32
    u32 = mybir.dt.uint32
    Op = mybir.AluOpType
    AX = mybir.AxisListType

    two_tables = d.get("TWO_TABLES", False)

    logits_h = nc.dram_tensor("logits", [rows, v], f32, kind="ExternalInput")
    gumbel_h = nc.dram_tensor("gumbel", [rows, v], f32, kind="ExternalInput")
    mask_h = nc.dram_tensor("mask", [rows, 1], i32, kind="ExternalInput")
    psg_h = nc.dram_tensor("psg", [rows, 1], i32, kind="ExternalInput")
    wte_h = nc.dram_tensor("wte", [v, e], f32, kind="ExternalInput")
    # the token branch reads W[:, col_map]; col_map is the identity here, so
    # both branches normally share one table (TWO_TABLES is a safety fallback)
    wtok_h = nc.dram_tensor("wte_tok", [v, e], f32, kind="ExternalInput") if two_tables else wte_h
    rowmap_h = nc.dram_tensor("rowmap", [v, 1], i32, kind="ExternalInput")
    tri_h = nc.dram_tensor("tri", [L, L], f32, kind="ExternalInput")
    out_h = nc.dram_tensor("out", [rows, e], f32, kind="ExternalOutput")
    # tiny passthrough pair so a benchmark can chain executions back-to-back
    chain_h = nc.dram_tensor("chain", [L, 8], f32, kind="ExternalInput")
    chain_o = nc.dram_tensor("chain_out", [L, 8], f32, kind="ExternalOutput")

    with tile.TileContext(nc) as tc:
        with (
            tc.tile_pool(name="lpool", bufs=lbufs) as lpool,
            tc.tile_pool(name="stats", bufs=2) as stats,
            tc.tile_pool(name="small", bufs=2) as small,
            tc.tile_pool(name="emb", bufs=2) as emb,
            tc.tile_pool(name="consts", bufs=1) as consts,
            tc.tile_pool(name="psum", bufs=2, space="PSUM") as psum,
        ):
            # ---- benchmark chain passthrough ----
            cht = consts.tile([L, 8], f32)
            nc.sync.dma_start(out=cht[:], in_=chain_h[:])
            nc.sync.dma_start(out=chain_o[:], in_=cht[:])

            # ---- per-core constants (built once) ----
            ones_mat = consts.tile([L, L], f32)
            nc.vector.memset(ones_mat[:], 1.0)
            tri_sb = consts.tile([L, L], f32)
            nc.sync.dma_start(out=tri_sb[:], in_=tri_h[:])

            iota_p_i = consts.tile([L, 1], i32)
            nc.gpsimd.iota(iota_p_i[:], pattern=[[1, 1]], base=0, channel_multiplier=1)
            iota_p = consts.tile([L, 1], f32)
            nc.vector.tensor_copy(out=iota_p[:], in_=iota_p_i[:])

            iota8_i = consts.tile([L, nsp], i32)
            nc.gpsimd.iota(iota8_i[:], pattern=[[1, nsp]], base=0, channel_multiplier=0)
            iota8 = consts.tile([L, nsp], f32)
            nc.vector.tensor_copy(out=iota8[:], in_=iota8_i[:])
            # c8rev[c] = nsp - c  (used to pick the LOWEST chunk that attains the max)
            c8rev = consts.tile([L, nsp], f32)
            nc.vector.tensor_scalar(c8rev[:], iota8[:], -1.0, float(nsp), op0=Op.mult, op1=Op.add)
            # per-chunk start offsets (hot = bases[c*] + within-chunk index)
            bases = consts.tile([L, nsp], f32)
            nc.vector.tensor_scalar(bases[:], iota8[:], float(ch), None, op0=Op.mult)
            for ci, (lo_c, _sz) in enumerate(spans):
                if lo_c != ci * ch:
                    nc.vector.memset(bases[:, ci:ci + 1], float(lo_c))

            def psg_phase(t):
                """Everything that does not depend on the streamed logits:
                mask/psg index arithmetic, flag, psg-embedding gather."""
                tok = slice(t * L, (t + 1) * L)
                mask_i = small.tile([L, 1], i32, tag="mask_i")
                nc.sync.dma_start(out=mask_i[:], in_=mask_h[tok, :])
                mask_f = small.tile([L, 1], f32, tag="mask_f")
                nc.vector.tensor_copy(out=mask_f[:], in_=mask_i[:])

                # s (broadcast to all partitions) = sum(mask) via ones matmul
                s_ps = psum.tile([L, 1], f32, tag="s_ps")
                nc.tensor.matmul(out=s_ps[:], lhsT=ones_mat[:], rhs=mask_f[:], start=True, stop=True)
                s_bc = small.tile([L, 1], f32, tag="s_bc")
                nc.vector.tensor_copy(out=s_bc[:], in_=s_ps[:])

                def mod_l(x_ap, lo_fix=True, hi_fix=True, tagp=""):
                    # x <- x mod L for x in (-L, 2L)
                    if hi_fix:
                        ge = small.tile([L, 1], f32, tag="ge" + tagp)
                        nc.vector.tensor_scalar(ge[:], x_ap, float(L), None, op0=Op.is_ge)
                        nc.vector.scalar_tensor_tensor(
                            out=x_ap, in0=ge[:], scalar=-float(L), in1=x_ap, op0=Op.mult, op1=Op.add)
                    if lo_fix:
                        lt_ = small.tile([L, 1], f32, tag="lt" + tagp)
                        nc.vector.tensor_scalar(lt_[:], x_ap, 0.0, None, op0=Op.is_lt)
                        nc.vector.scalar_tensor_tensor(
                            out=x_ap, in0=lt_[:], scalar=float(L), in1=x_ap, op0=Op.mult, op1=Op.add)

                # fidx = (L-1 + s - l) mod L   (flipped-mask gather index)
                fidx = small.tile([L, 1], f32, tag="fidx")
                nc.vector.scalar_tensor_tensor(
                    out=fidx[:], in0=s_bc[:], scalar=float(L - 1), in1=iota_p[:],
                    op0=Op.add, op1=Op.subtract)
                mod_l(fidx[:], lo_fix=False, tagp="f")

                # pidx = (L-1 - s + l) mod L   (rolled-psg gather index)
                pidx = small.tile([L, 1], f32, tag="pidx")
                nc.vector.scalar_tensor_tensor(
                    out=pidx[:], in0=s_bc[:], scalar=-1.0, in1=iota_p[:],
                    op0=Op.mult, op1=Op.add)
                nc.vector.tensor_scalar(pidx[:], pidx[:], float(L - 1), None, op0=Op.add)
                mod_l(pidx[:], tagp="p")

                # k = (l - s) mod L ; BOS position is k == 0
                kk = small.tile([L, 1], f32, tag="kk")
                nc.vector.scalar_tensor_tensor(
                    out=kk[:], in0=s_bc[:], scalar=-1.0, in1=iota_p[:],
                    op0=Op.mult, op1=Op.add)
                mod_l(kk[:], hi_fix=False, tagp="k")
                bos = small.tile([L, 1], f32, tag="bos")
                nc.vector.tensor_scalar(bos[:], kk[:], 0.0, None, op0=Op.is_equal)

                # gather mask[fidx] and psg[pidx] (within this batch row)
                fr_i = small.tile([L, 1], i32, tag="fr_i")
                nc.vector.tensor_scalar(fidx[:], fidx[:], float(t * L), None, op0=Op.add)
                nc.vector.tensor_copy(out=fr_i[:], in_=fidx[:])
                mrev = small.tile([L, 1], i32, tag="mrev")
                nc.gpsimd.indirect_dma_start(
                    out=mrev[:], out_offset=None, in_=mask_h[:],
                    in_offset=IndirectOffsetOnAxis(ap=fr_i[:, 0:1], axis=0),
                )
                pr_i = small.tile([L, 1], i32, tag="pr_i")
                nc.vector.tensor_scalar(pidx[:], pidx[:], float(t * L), None, op0=Op.add)
                nc.vector.tensor_copy(out=pr_i[:], in_=pidx[:])
                prot = small.tile([L, 1], i32, tag="prot")
                nc.gpsimd.indirect_dma_start(
                    out=prot[:], out_offset=None, in_=psg_h[:],
                    in_offset=IndirectOffsetOnAxis(ap=pr_i[:, 0:1], axis=0),
                )

                # f_rot = 1 - mask[fidx]
                mrev_f = small.tile([L, 1], f32, tag="mrev_f")
                nc.vector.tensor_copy(out=mrev_f[:], in_=mrev[:])
                frot = small.tile([L, 1], f32, tag="frot")
                nc.vector.tensor_scalar(frot[:], mrev_f[:], -1.0, 1.0, op0=Op.mult, op1=Op.add)
                # psg_rot = bos ? 1 : psg[pidx]
                prot_f = small.tile([L, 1], f32, tag="prot_f")
                nc.vector.tensor_copy(out=prot_f[:], in_=prot[:])
                nbos = small.tile([L, 1], f32, tag="nbos")
                nc.vector.tensor_scalar(nbos[:], bos[:], -1.0, 1.0, op0=Op.mult, op1=Op.add)
                nc.vector.tensor_tensor(out=prot_f[:], in0=prot_f[:], in1=nbos[:], op=Op.mult)
                nc.vector.tensor_tensor(out=prot_f[:], in0=prot_f[:], in1=bos[:], op=Op.add)
                # trunc = f_rot * psg_rot
                trunc = small.tile([L, 1], f32, tag="trunc")
                nc.vector.tensor_tensor(out=trunc[:], in0=frot[:], in1=prot_f[:], op=Op.mult)

                # flag = cumsum(trunc != 0) > 0 via triangular matmul
                nz = small.tile([L, 1], f32, tag="nz")
                nc.vector.tensor_scalar(nz[:], trunc[:], 0.0, None, op0=Op.not_equal)
                cnt_ps = psum.tile([L, 1], f32, tag="cnt_ps")
                nc.tensor.matmul(out=cnt_ps[:], lhsT=tri_sb[:], rhs=nz[:], start=True, stop=True)
                flag = small.tile([L, 1], f32, tag="flag")
                nc.vector.tensor_scalar(flag[:], cnt_ps[:], 0.0, None, op0=Op.is_gt)

                trunc_i = small.tile([L, 1], i32, tag="trunc_i")
                nc.vector.tensor_copy(out=trunc_i[:], in_=trunc[:])
                psgemb = emb.tile([L, e], f32, tag="psgemb")
                nc.gpsimd.indirect_dma_start(
                    out=psgemb[:], out_offset=None, in_=wte_h[:],
                    in_offset=IndirectOffsetOnAxis(ap=trunc_i[:, 0:1], axis=0),
                )
                return mask_f, flag, psgemb

            def stream_phase(t):
                """DMA-bound pass over the vocab: per chunk, load logits,
                accumulate gumbel in the DMA datapath, track max + argmax."""
                tok = slice(t * L, (t + 1) * L)
                m_all = stats.tile([L, nsp], f32, tag="m_all")
                idx_all = stats.tile([L, nsp], f32, tag="idx_all")
                for c, (lo, csz) in enumerate(spans):
                    lt = lpool.tile([L, ch], f32, tag="lt")
                    ldeng = nc.scalar if (d.get("DUAL_HWDGE", True) and c % 2) else nc.sync
                    ldeng.dma_start(out=lt[:, 0:csz], in_=logits_h[tok, lo:lo + csz])
                    if add_mode == "accum":
                        # s = logits + gumbel via DMA CCE inline add;
                        # descriptors must stay <= 2048 elements each.
                        half = ch // 2
                        if not skip_accum and ch <= 2048:
                            nc.gpsimd.dma_start(
                                out=lt[:, 0:csz], in_=gumbel_h[tok, lo:lo + csz],
                                accum_op=Op.add)
                        elif not skip_accum:
                            if d.get("ACCUM3D", False):
                                gsrc = gumbel_h[tok, lo:lo + ch].rearrange(
                                    "p (a b) -> p a b", b=half)
                                ldst = lt[:].rearrange("p (a b) -> p a b", b=half)
                                nc.gpsimd.dma_start(out=ldst, in_=gsrc, accum_op=Op.add)
                            else:
                                nc.gpsimd.dma_start(
                                    out=lt[:, 0:half], in_=gumbel_h[tok, lo:lo + half],
                                    accum_op=Op.add)
                                nc.gpsimd.dma_start(
                                    out=lt[:, half:ch], in_=gumbel_h[tok, lo + half:lo + ch],
                                    accum_op=Op.add)
                    elif add_mode == "hybrid":
                        # half the gumbel chunk via SWDGE inline-add DMA,
                        # half via HWDGE load + DVE add: balances Pool.SEQ
                        # descriptor emission against DVE cycles.
                        half = ch // 2
                        nc.gpsimd.dma_start(
                            out=lt[:, 0:half], in_=gumbel_h[tok, lo:lo + half],
                            accum_op=Op.add)
                        gt = lpool.tile([L, half], f32, tag="gt")
                        nc.sync.dma_start(out=gt[:], in_=gumbel_h[tok, lo + half:lo + ch])
                        nc.vector.tensor_tensor(out=lt[:, half:ch], in0=lt[:, half:ch], in1=gt[:], op=Op.add)
                    else:
                        gt = lpool.tile([L, ch], f32, tag="gt")
                        nc.sync.dma_start(out=gt[:], in_=gumbel_h[tok, lo:lo + ch])
                        eng = nc.vector if (add_mode == "dve" or c % 2 == 0) else nc.gpsimd
                        eng.tensor_tensor(out=lt[:], in0=lt[:], in1=gt[:], op=Op.add)
                    # chunk max + within-chunk argmax (first occurrence);
                    # the column copies go to the otherwise-idle ACT engine
                    mx8 = small.tile([L, 8], f32, tag="mx8")
                    nc.vector.max(out=mx8[:], in_=lt[:, 0:csz])
                    if smalls_on_act:
                        nc.scalar.copy(out=m_all[:, c:c + 1], in_=mx8[:, 0:1])
                    else:
                        nc.vector.tensor_copy(out=m_all[:, c:c + 1], in_=mx8[:, 0:1])
                    mi8 = small.tile([L, 8], u32, tag="mi8")
                    if not skip_maxidx:
                        nc.vector.max_index(out=mi8[:], in_max=mx8[:], in_values=lt[:, 0:csz])
                    else:
                        nc.vector.memset(mi8[:], 0)
                    if smalls_on_act:
                        nc.scalar.copy(out=idx_all[:, c:c + 1], in_=mi8[:, 0:1])
                    else:
                        nc.vector.tensor_copy(out=idx_all[:, c:c + 1], in_=mi8[:, 0:1])
                return m_all, idx_all

            def tail_phase(t, m_all, idx_all, mask_f, flag, psgemb):
                tok = slice(t * L, (t + 1) * L)
                # global max + first chunk attaining it
                gmax = small.tile([L, 1], f32, tag="gmax")
                nc.vector.reduce_max(out=gmax[:], in_=m_all[:], axis=AX.X)
                sel8 = small.tile([L, nsp], f32, tag="sel8")
                nc.vector.scalar_tensor_tensor(
                    out=sel8[:], in0=m_all[:], scalar=gmax[:, 0:1], in1=c8rev[:],
                    op0=Op.is_ge, op1=Op.mult)
                cmax = small.tile([L, 1], f32, tag="cmax")
                nc.vector.reduce_max(out=cmax[:], in_=sel8[:], axis=AX.X)
                cstar = small.tile([L, 1], f32, tag="cstar")
                nc.vector.tensor_scalar(cstar[:], cmax[:], -1.0, float(nsp), op0=Op.mult, op1=Op.add)
                # winning chunk's within-chunk index and base offset
                junk8 = small.tile([L, nsp], f32, tag="junk8")
                nc.vector.scalar_tensor_tensor(
                    out=junk8[:], in0=iota8[:], scalar=cstar[:, 0:1], in1=idx_all[:],
                    op0=Op.is_equal, op1=Op.mult)
                mi_sel = small.tile([L, 1], f32, tag="mi_sel")
                nc.vector.reduce_max(out=mi_sel[:], in_=junk8[:], axis=AX.X)
                junk8b = small.tile([L, nsp], f32, tag="junk8b")
                nc.vector.scalar_tensor_tensor(
                    out=junk8b[:], in0=iota8[:], scalar=cstar[:, 0:1], in1=bases[:],
                    op0=Op.is_equal, op1=Op.mult)
                base_sel = small.tile([L, 1], f32, tag="base_sel")
                nc.vector.reduce_max(out=base_sel[:], in_=junk8b[:], axis=AX.X)
                hot_f = small.tile([L, 1], f32, tag="hot_f")
                nc.vector.tensor_tensor(out=hot_f[:], in0=base_sel[:], in1=mi_sel[:], op=Op.add)
                hot_i = small.tile([L, 1], i32, tag="hot_i")
                nc.vector.tensor_copy(out=hot_i[:], in_=hot_f[:])

                # hot -> vocab row (grid_sample LUT), -> token embeddings
                rowidx = small.tile([L, 1], i32, tag="rowidx")
                nc.gpsimd.indirect_dma_start(
                    out=rowidx[:], out_offset=None, in_=rowmap_h[:],
                    in_offset=IndirectOffsetOnAxis(ap=hot_i[:, 0:1], axis=0),
                )
                tokemb = emb.tile([L, e], f32, tag="tokemb")
                nc.gpsimd.indirect_dma_start(
                    out=tokemb[:], out_offset=None, in_=wtok_h[:],
                    in_offset=IndirectOffsetOnAxis(ap=rowidx[:, 0:1], axis=0),
                )

                # combine + store
                p1 = emb.tile([L, e], f32, tag="p1")
                nc.vector.tensor_scalar(p1[:], tokemb[:], mask_f[:, 0:1], None, op0=Op.mult)
                outt = emb.tile([L, e], f32, tag="outt")
                nc.vector.scalar_tensor_tensor(
                    out=outt[:], in0=psgemb[:], scalar=flag[:, 0:1], in1=p1[:],
                    op0=Op.mult, op1=Op.add)
                nc.sync.dma_start(out=out_h[tok, :], in_=outt[:])

            for _ in range(body_reps):
                if skip_tail:
                    for t in range(b_loc):
                        m_all, idx_all = stream_phase(t)
                        tok = slice(t * L, (t + 1) * L)
                        dummy = emb.tile([L, e], f32, tag="outt")
                        nc.vector.tensor_scalar(dummy[:], m_all[:, 0:1].to_broadcast([L, e]), 1.0, None, op0=Op.mult)
                        nc.sync.dma_start(out=out_h[tok, :], in_=dummy[:])
                    continue
                psg_state = [psg_phase(t) for t in range(b_loc)]
                if tail_after_each:
                    for t in range(b_loc):
                        m_all, idx_all = stream_phase(t)
                        tail_phase(t, m_all, idx_all, *psg_state[t])
                else:
                    streams = [stream_phase(t) for t in range(b_loc)]
                    for t in range(b_loc):
                        tail_phase(t, *streams[t], *psg_state[t])

    return nc


_BUILD_CACHE = {}


def _get_module(dims_key=None, dims=None, body_reps=1):
    key = (dims_key, body_reps)
    if key not in _BUILD_CACHE:
        import concourse.bacc as bacc

        nc = bacc.Bacc("TRN2", target_bir_lowering=False, debug=False)
        _build(nc, dims, body_reps=body_reps)
        nc.compile()
        _BUILD_CACHE[key] = nc
    return _BUILD_CACHE[key]


_ROWMAP_CACHE = {}


def _nearest_maps():
    """Replicate the reference's f32 grid_sample-nearest index maps with jnp
    on the same backend the reference runs on (bit-exact by construction)."""
    if "maps" not in _ROWMAP_CACHE:
        import jax.numpy as jnp

        def nearest(size):
            lin = jnp.linspace(-1.0, 1.0, size)
            ix = ((lin + 1.0) * size - 1.0) / 2.0
            return np.asarray(jnp.clip(jnp.round(ix), 0, size - 1).astype(jnp.int32))

        _ROWMAP_CACHE["maps"] = (nearest(V), nearest(E))
    return _ROWMAP_CACHE["maps"]


_TRI = None

# test/dev hooks: set TRACE=True before calling kernel() to capture an NTFF
# profile; the BassKernelResults of the last run is stored in LAST_RESULT.
TRACE = False
LAST_RESULT = None


def kernel(logits, rwrt_attention_mask, psg_input_ids, word_embeddings, gumbel_noise):
    from concourse.bass_utils import run_bass_kernel_spmd

    global _TRI
    logits = np.ascontiguousarray(np.asarray(logits, dtype=np.float32))
    gumbel = np.ascontiguousarray(np.asarray(gumbel_noise, dtype=np.float32))
    mask = np.ascontiguousarray(np.asarray(rwrt_attention_mask, dtype=np.int32))
    psg = np.ascontiguousarray(np.asarray(psg_input_ids, dtype=np.int32))
    wte = np.ascontiguousarray(np.asarray(word_embeddings, dtype=np.float32))

    rowmap, colmap = _nearest_maps()
    col_identity = bool(np.array_equal(colmap, np.arange(E, dtype=np.int32)))
    rowmap2 = rowmap.reshape(V, 1)
    if _TRI is None:
        _TRI = np.ascontiguousarray(np.triu(np.ones((L, L), dtype=np.float32)))

    if col_identity:
        nc = _get_module()
    else:
        # safety fallback (never taken in this environment): bake the column
        # permutation into a separate token-branch table
        nc = _get_module(dims_key="two_tables", dims={"TWO_TABLES": True})
        wte_tok = np.ascontiguousarray(wte[:, colmap])

    in_maps = []
    for m in range(N_CORES):
        sl = slice(m * B_LOC, (m + 1) * B_LOC)
        im = {
            "logits": logits[sl].reshape(B_LOC * L, V),
            "gumbel": gumbel[sl].reshape(B_LOC * L, V),
            "mask": mask[sl].reshape(B_LOC * L, 1),
            "psg": psg[sl].reshape(B_LOC * L, 1),
            "wte": wte,
            "rowmap": rowmap2,
            "tri": _TRI,
            "chain": np.zeros((L, 8), np.float32),
        }
        if not col_identity:
            im["wte_tok"] = wte_tok
        in_maps.append(im)

    global LAST_RESULT
    try:
        LAST_RESULT = run_bass_kernel_spmd(nc, in_maps, list(range(N_CORES)), trace=TRACE)
    except Exception:
        # the axon-relayed device occasionally reports a transient
        # NRT_EXEC_UNIT_UNRECOVERABLE on the first execution after long
        # sessions; a straight re-run recovers it
        import time as _time

        _time.sleep(2.0)
        LAST_RESULT = run_bass_kernel_spmd(nc, in_maps, list(range(N_CORES)), trace=TRACE)
    res = LAST_RESULT.results
    out = np.concatenate(
        [res[m]["out"].reshape(B_LOC, L, E) for m in range(N_CORES)], axis=0
    )
    return out


# revision 41
# speedup vs baseline: 1.0483x; 1.0033x over previous
"""Trainium2 Bass kernel: gumbel-softmax-argmax embedding lookup (end-to-end).

Reference math (nn_End2End_49495203119139):
    hot  = argmax_V(softmax((logits + gumbel)/tau))       == argmax_V(logits+gumbel)
    row  = grid_sample-nearest index map of hot            == ROWMAP[hot]  (LUT)
    tok_emb = W[row][:, col_map]   with col_map == arange(E)  (verified at runtime)
    inputs_embeds = tok_emb * mask
    psg_roll = roll(psg_ids, 1, axis=1); psg_roll[:,0] = 1
    extr  = (1 - mask[:, ::-1]) * psg_roll
    trunc = rotate_right(extr, shifts) with shifts = mask.sum(-1)   (per row)
    flag  = cumsum(trunc != 0, -1) > 0
    out   = inputs_embeds + where(flag, W[trunc], 0)

Sharding: data-parallel over batch. B=16 over 8 cores -> 2 batch rows
(= 2 token tiles of 128) per core; the embedding table is replicated.

Per-core device plan (memory-bound part = streaming logits+gumbel, 66 MB,
~184 us HBM floor at ~358 GB/s per core):
  - for each token tile (128 tokens on partitions) and each vocab chunk
    [128 x 2008]: HWDGE-load the logits chunk, then add the gumbel chunk
    with one SWDGE inline-accumulate DMA (CCE add; descriptors must stay
    <= 2048 elements — larger accumulates crash the device).
    DVE `max` finds the chunk max, `max_index` the first within-chunk
    argmax position (ties resolve to the lowest index, matching argmax).
  - chunk winner (lowest chunk attaining the global max) + within-chunk
    index give `hot`; ROWMAP and W rows come via indirect DMA gathers.
  - the passage branch is pure index arithmetic on [128,1] tiles: the
    reverse/roll/rotate are folded into gather indices modulo L, the
    mask-sum and cumsum are 0/1 matmuls against ones/triangular matrices
    (exact in any PE precision).
Predicted 212.4 us/core by the TimelineSim cost model (DMA engines busy
190 us of that, i.e. ~97% of the 66MB/358GB/s floor); a hardware
min-slope measurement of the 4016-chunk variant gave ~204 us.
"""

import numpy as np

B = 16
L = 128
V = 32128
E = 768
N_CORES = 8
B_LOC = B // N_CORES          # batch rows per core
CH = 2008                     # vocab chunk (free dim) per streamed tile;
                              # <= 2048 so a gumbel chunk is ONE CCE-add DMA
NCH = V // CH                 # 16 chunks
NEG = -3.0e38


def _build(nc_mod, dims=None, body_reps=1):
    """Build the per-core Bass module. dims allows small smoke-test builds;
    body_reps>1 repeats the whole body (for slope-based benchmarking)."""
    import concourse.tile as tile
    from concourse import bass, mybir
    from concourse.bass import IndirectOffsetOnAxis

    d = dims or {}
    v = d.get("V", V)
    e = d.get("E", E)
    ch = d.get("CH", CH)
    nch = v // ch
    b_loc = d.get("B_LOC", B_LOC)
    rows = b_loc * L
    lbufs = d.get("LBUFS", 8)
    skip_tail = d.get("SKIP_TAIL", False)
    skip_accum = d.get("SKIP_ACCUM", False)
    skip_maxidx = d.get("SKIP_MAXIDX", False)
    tail_after_each = d.get("TAIL_AFTER_EACH", False)
    # how logits+gumbel are summed: "accum" = DMA CCE inline add (SWDGE),
    # "dve" = plain loads + DVE adds, "split" = adds alternate DVE/GpSimd
    add_mode = d.get("ADD_MODE", "accum")
    # chunk spans (lo, size); SMALL_LAST splits the final chunk so the
    # post-last-DMA DVE chain (max+max_index of the last chunk) is short
    spans = [(c * ch, ch) for c in range(nch)]
    if d.get("SMALL_LAST", False) and ch >= 1024:
        lo_last, sz = spans.pop()
        spans.append((lo_last, sz - 502))
        spans.append((lo_last + sz - 502, 502))
    nsp = len(spans)
    max_eng = d.get("MAX_ENG", "dve")
    smalls_on_act = d.get("SMALLS_ON_ACT", False)

    nc = nc_mod
    f32 = mybir.dt.float32
    i32 = mybir.dt.int32
    u32 = mybir.dt.uint32
    Op = mybir.AluOpType
    AX = mybir.AxisListType

    two_tables = d.get("TWO_TABLES", False)

    logits_h = nc.dram_tensor("logits", [rows, v], f32, kind="ExternalInput")
    gumbel_h = nc.dram_tensor("gumbel", [rows, v], f32, kind="ExternalInput")
    mask_h = nc.dram_tensor("mask", [rows, 1], i32, kind="ExternalInput")
    psg_h = nc.dram_tensor("psg", [rows, 1], i32, kind="ExternalInput")
    wte_h = nc.dram_tensor("wte", [v, e], f32, kind="ExternalInput")
    # the token branch reads W[:, col_map]; col_map is the identity here, so
    # both branches normally share one table (TWO_TABLES is a safety fallback)
    wtok_h = nc.dram_tensor("wte_tok", [v, e], f32, kind="ExternalInput") if two_tables else wte_h
    rowmap_h = nc.dram_tensor("rowmap", [v, 1], i32, kind="ExternalInput")
    tri_h = nc.dram_tensor("tri", [L, L], f32, kind="ExternalInput")
    out_h = nc.dram_tensor("out", [rows, e], f32, kind="ExternalOutput")
    # tiny passthrough pair so a benchmark can chain executions back-to-back
    chain_h = nc.dram_tensor("chain", [L, 8], f32, kind="ExternalInput")
    chain_o = nc.dram_tensor("chain_out", [L, 8], f32, kind="ExternalOutput")

    with tile.TileContext(nc) as tc:
        with (
            tc.tile_pool(name="lpool", bufs=lbufs) as lpool,
            tc.tile_pool(name="stats", bufs=2) as stats,
            tc.tile_pool(name="small", bufs=2) as small,
            tc.tile_pool(name="emb", bufs=2) as emb,
            tc.tile_pool(name="consts", bufs=1) as consts,
            tc.tile_pool(name="psum", bufs=2, space="PSUM") as psum,
        ):
            # ---- benchmark chain passthrough ----
            cht = consts.tile([L, 8], f32)
            nc.scalar.dma_start(out=cht[:], in_=chain_h[:])
            nc.scalar.dma_start(out=chain_o[:], in_=cht[:])

            # ---- per-core constants (built once) ----
            ones_mat = consts.tile([L, L], f32)
            nc.vector.memset(ones_mat[:], 1.0)
            tri_sb = consts.tile([L, L], f32)
            nc.scalar.dma_start(out=tri_sb[:], in_=tri_h[:])

            iota_p_i = consts.tile([L, 1], i32)
            nc.gpsimd.iota(iota_p_i[:], pattern=[[1, 1]], base=0, channel_multiplier=1)
            iota_p = consts.tile([L, 1], f32)
            nc.vector.tensor_copy(out=iota_p[:], in_=iota_p_i[:])

            iota8_i = consts.tile([L, nsp], i32)
            nc.gpsimd.iota(iota8_i[:], pattern=[[1, nsp]], base=0, channel_multiplier=0)
            iota8 = consts.tile([L, nsp], f32)
            nc.vector.tensor_copy(out=iota8[:], in_=iota8_i[:])
            # c8rev[c] = nsp - c  (used to pick the LOWEST chunk that attains the max)
            c8rev = consts.tile([L, nsp], f32)
            nc.vector.tensor_scalar(c8rev[:], iota8[:], -1.0, float(nsp), op0=Op.mult, op1=Op.add)
            # per-chunk start offsets (hot = bases[c*] + within-chunk index)
            bases = consts.tile([L, nsp], f32)
            nc.vector.tensor_scalar(bases[:], iota8[:], float(ch), None, op0=Op.mult)
            for ci, (lo_c, _sz) in enumerate(spans):
                if lo_c != ci * ch:
                    nc.vector.memset(bases[:, ci:ci + 1], float(lo_c))

            def psg_phase(t):
                """Everything that does not depend on the streamed logits:
                mask/psg index arithmetic, flag, psg-embedding gather."""
                tok = slice(t * L, (t + 1) * L)
                mask_i = small.tile([L, 1], i32, tag="mask_i")
                nc.scalar.dma_start(out=mask_i[:], in_=mask_h[tok, :])
                mask_f = small.tile([L, 1], f32, tag="mask_f")
                nc.vector.tensor_copy(out=mask_f[:], in_=mask_i[:])

                # s (broadcast to all partitions) = sum(mask) via ones matmul
                s_ps = psum.tile([L, 1], f32, tag="s_ps")
                nc.tensor.matmul(out=s_ps[:], lhsT=ones_mat[:], rhs=mask_f[:], start=True, stop=True)
                s_bc = small.tile([L, 1], f32, tag="s_bc")
                nc.vector.tensor_copy(out=s_bc[:], in_=s_ps[:])

                def mod_l(x_ap, lo_fix=True, hi_fix=True, tagp=""):
                    # x <- x mod L for x in (-L, 2L)
                    if hi_fix:
                        ge = small.tile([L, 1], f32, tag="ge" + tagp)
                        nc.vector.tensor_scalar(ge[:], x_ap, float(L), None, op0=Op.is_ge)
                        nc.vector.scalar_tensor_tensor(
                            out=x_ap, in0=ge[:], scalar=-float(L), in1=x_ap, op0=Op.mult, op1=Op.add)
                    if lo_fix:
                        lt_ = small.tile([L, 1], f32, tag="lt" + tagp)
                        nc.vector.tensor_scalar(lt_[:], x_ap, 0.0, None, op0=Op.is_lt)
                        nc.vector.scalar_tensor_tensor(
                            out=x_ap, in0=lt_[:], scalar=float(L), in1=x_ap, op0=Op.mult, op1=Op.add)

                # fidx = (L-1 + s - l) mod L   (flipped-mask gather index)
                fidx = small.tile([L, 1], f32, tag="fidx")
                nc.vector.scalar_tensor_tensor(
                    out=fidx[:], in0=s_bc[:], scalar=float(L - 1), in1=iota_p[:],
                    op0=Op.add, op1=Op.subtract)
                mod_l(fidx[:], lo_fix=False, tagp="f")

                # pidx = (L-1 - s + l) mod L   (rolled-psg gather index)
                pidx = small.tile([L, 1], f32, tag="pidx")
                nc.vector.scalar_tensor_tensor(
                    out=pidx[:], in0=s_bc[:], scalar=-1.0, in1=iota_p[:],
                    op0=Op.mult, op1=Op.add)
                nc.vector.tensor_scalar(pidx[:], pidx[:], float(L - 1), None, op0=Op.add)
                mod_l(pidx[:], tagp="p")

                # k = (l - s) mod L ; BOS position is k == 0
                kk = small.tile([L, 1], f32, tag="kk")
                nc.vector.scalar_tensor_tensor(
                    out=kk[:], in0=s_bc[:], scalar=-1.0, in1=iota_p[:],
                    op0=Op.mult, op1=Op.add)
                mod_l(kk[:], hi_fix=False, tagp="k")
                bos = small.tile([L, 1], f32, tag="bos")
                nc.vector.tensor_scalar(bos[:], kk[:], 0.0, None, op0=Op.is_equal)

                # gather mask[fidx] and psg[pidx] (within this batch row)
                fr_i = small.tile([L, 1], i32, tag="fr_i")
                nc.vector.tensor_scalar(fidx[:], fidx[:], float(t * L), None, op0=Op.add)
                nc.vector.tensor_copy(out=fr_i[:], in_=fidx[:])
                mrev = small.tile([L, 1], i32, tag="mrev")
                nc.gpsimd.indirect_dma_start(
                    out=mrev[:], out_offset=None, in_=mask_h[:],
                    in_offset=IndirectOffsetOnAxis(ap=fr_i[:, 0:1], axis=0),
                )
                pr_i = small.tile([L, 1], i32, tag="pr_i")
                nc.vector.tensor_scalar(pidx[:], pidx[:], float(t * L), None, op0=Op.add)
                nc.vector.tensor_copy(out=pr_i[:], in_=pidx[:])
                prot = small.tile([L, 1], i32, tag="prot")
                nc.gpsimd.indirect_dma_start(
                    out=prot[:], out_offset=None, in_=psg_h[:],
                    in_offset=IndirectOffsetOnAxis(ap=pr_i[:, 0:1], axis=0),
                )

                # f_rot = 1 - mask[fidx]
                mrev_f = small.tile([L, 1], f32, tag="mrev_f")
                nc.vector.tensor_copy(out=mrev_f[:], in_=mrev[:])
                frot = small.tile([L, 1], f32, tag="frot")
                nc.vector.tensor_scalar(frot[:], mrev_f[:], -1.0, 1.0, op0=Op.mult, op1=Op.add)
                # psg_rot = bos ? 1 : psg[pidx]
                prot_f = small.tile([L, 1], f32, tag="prot_f")
                nc.vector.tensor_copy(out=prot_f[:], in_=prot[:])
                nbos = small.tile([L, 1], f32, tag="nbos")
                nc.vector.tensor_scalar(nbos[:], bos[:], -1.0, 1.0, op0=Op.mult, op1=Op.add)
                nc.vector.tensor_tensor(out=prot_f[:], in0=prot_f[:], in1=nbos[:], op=Op.mult)
                nc.vector.tensor_tensor(out=prot_f[:], in0=prot_f[:], in1=bos[:], op=Op.add)
                # trunc = f_rot * psg_rot
                trunc = small.tile([L, 1], f32, tag="trunc")
                nc.vector.tensor_tensor(out=trunc[:], in0=frot[:], in1=prot_f[:], op=Op.mult)

                # flag = cumsum(trunc != 0) > 0 via triangular matmul
                nz = small.tile([L, 1], f32, tag="nz")
                nc.vector.tensor_scalar(nz[:], trunc[:], 0.0, None, op0=Op.not_equal)
                cnt_ps = psum.tile([L, 1], f32, tag="cnt_ps")
                nc.tensor.matmul(out=cnt_ps[:], lhsT=tri_sb[:], rhs=nz[:], start=True, stop=True)
                flag = small.tile([L, 1], f32, tag="flag")
                nc.vector.tensor_scalar(flag[:], cnt_ps[:], 0.0, None, op0=Op.is_gt)

                trunc_i = small.tile([L, 1], i32, tag="trunc_i")
                nc.vector.tensor_copy(out=trunc_i[:], in_=trunc[:])
                psgemb = emb.tile([L, e], f32, tag="psgemb")
                nc.gpsimd.indirect_dma_start(
                    out=psgemb[:], out_offset=None, in_=wte_h[:],
                    in_offset=IndirectOffsetOnAxis(ap=trunc_i[:, 0:1], axis=0),
                )
                return mask_f, flag, psgemb

            def stream_phase(t):
                """DMA-bound pass over the vocab: per chunk, load logits,
                accumulate gumbel in the DMA datapath, track max + argmax."""
                tok = slice(t * L, (t + 1) * L)
                m_all = stats.tile([L, nsp], f32, tag="m_all")
                idx_all = stats.tile([L, nsp], f32, tag="idx_all")
                for c, (lo, csz) in enumerate(spans):
                    lt = lpool.tile([L, ch], f32, tag="lt")
                    ldeng = nc.scalar if (d.get("DUAL_HWDGE", True) and c % 2) else nc.sync
                    ldeng.dma_start(out=lt[:, 0:csz], in_=logits_h[tok, lo:lo + csz])
                    if add_mode == "accum":
                        # s = logits + gumbel via DMA CCE inline add;
                        # descriptors must stay <= 2048 elements each.
                        half = ch // 2
                        if not skip_accum and ch <= 2048:
                            nc.gpsimd.dma_start(
                                out=lt[:, 0:csz], in_=gumbel_h[tok, lo:lo + csz],
                                accum_op=Op.add)
                        elif not skip_accum:
                            if d.get("ACCUM3D", False):
                                gsrc = gumbel_h[tok, lo:lo + ch].rearrange(
                                    "p (a b) -> p a b", b=half)
                                ldst = lt[:].rearrange("p (a b) -> p a b", b=half)
                                nc.gpsimd.dma_start(out=ldst, in_=gsrc, accum_op=Op.add)
                            else:
                                nc.gpsimd.dma_start(
                                    out=lt[:, 0:half], in_=gumbel_h[tok, lo:lo + half],
                                    accum_op=Op.add)
                                nc.gpsimd.dma_start(
                                    out=lt[:, half:ch], in_=gumbel_h[tok, lo + half:lo + ch],
                                    accum_op=Op.add)
                    elif add_mode == "hybrid":
                        # half the gumbel chunk via SWDGE inline-add DMA,
                        # half via HWDGE load + DVE add: balances Pool.SEQ
                        # descriptor emission against DVE cycles.
                        half = ch // 2
                        nc.gpsimd.dma_start(
                            out=lt[:, 0:half], in_=gumbel_h[tok, lo:lo + half],
                            accum_op=Op.add)
                        gt = lpool.tile([L, half], f32, tag="gt")
                        nc.sync.dma_start(out=gt[:], in_=gumbel_h[tok, lo + half:lo + ch])
                        nc.vector.tensor_tensor(out=lt[:, half:ch], in0=lt[:, half:ch], in1=gt[:], op=Op.add)
                    else:
                        gt = lpool.tile([L, ch], f32, tag="gt")
                        nc.sync.dma_start(out=gt[:], in_=gumbel_h[tok, lo:lo + ch])
                        eng = nc.vector if (add_mode == "dve" or c % 2 == 0) else nc.gpsimd
                        eng.tensor_tensor(out=lt[:], in0=lt[:], in1=gt[:], op=Op.add)
                    # chunk max + within-chunk argmax (first occurrence);
                    # the column copies go to the otherwise-idle ACT engine
                    mx8 = small.tile([L, 8], f32, tag="mx8")
                    nc.vector.max(out=mx8[:], in_=lt[:, 0:csz])
                    if smalls_on_act:
                        nc.scalar.copy(out=m_all[:, c:c + 1], in_=mx8[:, 0:1])
                    else:
                        nc.vector.tensor_copy(out=m_all[:, c:c + 1], in_=mx8[:, 0:1])
                    mi8 = small.tile([L, 8], u32, tag="mi8")
                    if not skip_maxidx:
                        nc.vector.max_index(out=mi8[:], in_max=mx8[:], in_values=lt[:, 0:csz])
                    else:
                        nc.vector.memset(mi8[:], 0)
                    if smalls_on_act:
                        nc.scalar.copy(out=idx_all[:, c:c + 1], in_=mi8[:, 0:1])
                    else:
                        nc.vector.tensor_copy(out=idx_all[:, c:c + 1], in_=mi8[:, 0:1])
                return m_all, idx_all

            def tail_phase(t, m_all, idx_all, mask_f, flag, psgemb):
                tok = slice(t * L, (t + 1) * L)
                # global max + first chunk attaining it
                gmax = small.tile([L, 1], f32, tag="gmax")
                nc.vector.reduce_max(out=gmax[:], in_=m_all[:], axis=AX.X)
                sel8 = small.tile([L, nsp], f32, tag="sel8")
                nc.vector.scalar_tensor_tensor(
                    out=sel8[:], in0=m_all[:], scalar=gmax[:, 0:1], in1=c8rev[:],
                    op0=Op.is_ge, op1=Op.mult)
                cmax = small.tile([L, 1], f32, tag="cmax")
                nc.vector.reduce_max(out=cmax[:], in_=sel8[:], axis=AX.X)
                cstar = small.tile([L, 1], f32, tag="cstar")
                nc.vector.tensor_scalar(cstar[:], cmax[:], -1.0, float(nsp), op0=Op.mult, op1=Op.add)
                # winning chunk's within-chunk index and base offset
                junk8 = small.tile([L, nsp], f32, tag="junk8")
                nc.vector.scalar_tensor_tensor(
                    out=junk8[:], in0=iota8[:], scalar=cstar[:, 0:1], in1=idx_all[:],
                    op0=Op.is_equal, op1=Op.mult)
                mi_sel = small.tile([L, 1], f32, tag="mi_sel")
                nc.vector.reduce_max(out=mi_sel[:], in_=junk8[:], axis=AX.X)
                junk8b = small.tile([L, nsp], f32, tag="junk8b")
                nc.vector.scalar_tensor_tensor(
                    out=junk8b[:], in0=iota8[:], scalar=cstar[:, 0:1], in1=bases[:],
                    op0=Op.is_equal, op1=Op.mult)
                base_sel = small.tile([L, 1], f32, tag="base_sel")
                nc.vector.reduce_max(out=base_sel[:], in_=junk8b[:], axis=AX.X)
                hot_f = small.tile([L, 1], f32, tag="hot_f")
                nc.vector.tensor_tensor(out=hot_f[:], in0=base_sel[:], in1=mi_sel[:], op=Op.add)
                hot_i = small.tile([L, 1], i32, tag="hot_i")
                nc.vector.tensor_copy(out=hot_i[:], in_=hot_f[:])

                # hot -> vocab row (grid_sample LUT), -> token embeddings
                rowidx = small.tile([L, 1], i32, tag="rowidx")
                nc.gpsimd.indirect_dma_start(
                    out=rowidx[:], out_offset=None, in_=rowmap_h[:],
                    in_offset=IndirectOffsetOnAxis(ap=hot_i[:, 0:1], axis=0),
                )
                tokemb = emb.tile([L, e], f32, tag="tokemb")
                nc.gpsimd.indirect_dma_start(
                    out=tokemb[:], out_offset=None, in_=wtok_h[:],
                    in_offset=IndirectOffsetOnAxis(ap=rowidx[:, 0:1], axis=0),
                )

                # combine + store
                p1 = emb.tile([L, e], f32, tag="p1")
                nc.vector.tensor_scalar(p1[:], tokemb[:], mask_f[:, 0:1], None, op0=Op.mult)
                outt = emb.tile([L, e], f32, tag="outt")
                nc.vector.scalar_tensor_tensor(
                    out=outt[:], in0=psgemb[:], scalar=flag[:, 0:1], in1=p1[:],
                    op0=Op.mult, op1=Op.add)
                nc.sync.dma_start(out=out_h[tok, :], in_=outt[:])

            for _ in range(body_reps):
                if skip_tail:
                    for t in range(b_loc):
                        m_all, idx_all = stream_phase(t)
                        tok = slice(t * L, (t + 1) * L)
                        dummy = emb.tile([L, e], f32, tag="outt")
                        nc.vector.tensor_scalar(dummy[:], m_all[:, 0:1].to_broadcast([L, e]), 1.0, None, op0=Op.mult)
                        nc.sync.dma_start(out=out_h[tok, :], in_=dummy[:])
                    continue
                psg_state = [psg_phase(t) for t in range(b_loc)]
                if tail_after_each:
                    for t in range(b_loc):
                        m_all, idx_all = stream_phase(t)
                        tail_phase(t, m_all, idx_all, *psg_state[t])
                else:
                    streams = [stream_phase(t) for t in range(b_loc)]
                    for t in range(b_loc):
                        tail_phase(t, *streams[t], *psg_state[t])

    return nc


_BUILD_CACHE = {}


def _get_module(dims_key=None, dims=None, body_reps=1):
    key = (dims_key, body_reps)
    if key not in _BUILD_CACHE:
        import concourse.bacc as bacc

        nc = bacc.Bacc("TRN2", target_bir_lowering=False, debug=False)
        _build(nc, dims, body_reps=body_reps)
        nc.compile()
        _BUILD_CACHE[key] = nc
    return _BUILD_CACHE[key]


_ROWMAP_CACHE = {}


def _nearest_maps():
    """Replicate the reference's f32 grid_sample-nearest index maps with jnp
    on the same backend the reference runs on (bit-exact by construction)."""
    if "maps" not in _ROWMAP_CACHE:
        import jax.numpy as jnp

        def nearest(size):
            lin = jnp.linspace(-1.0, 1.0, size)
            ix = ((lin + 1.0) * size - 1.0) / 2.0
            return np.asarray(jnp.clip(jnp.round(ix), 0, size - 1).astype(jnp.int32))

        _ROWMAP_CACHE["maps"] = (nearest(V), nearest(E))
    return _ROWMAP_CACHE["maps"]


_TRI = None

# test/dev hooks: set TRACE=True before calling kernel() to capture an NTFF
# profile; the BassKernelResults of the last run is stored in LAST_RESULT.
TRACE = False
LAST_RESULT = None


def kernel(logits, rwrt_attention_mask, psg_input_ids, word_embeddings, gumbel_noise):
    from concourse.bass_utils import run_bass_kernel_spmd

    global _TRI
    logits = np.ascontiguousarray(np.asarray(logits, dtype=np.float32))
    gumbel = np.ascontiguousarray(np.asarray(gumbel_noise, dtype=np.float32))
    mask = np.ascontiguousarray(np.asarray(rwrt_attention_mask, dtype=np.int32))
    psg = np.ascontiguousarray(np.asarray(psg_input_ids, dtype=np.int32))
    wte = np.ascontiguousarray(np.asarray(word_embeddings, dtype=np.float32))

    rowmap, colmap = _nearest_maps()
    col_identity = bool(np.array_equal(colmap, np.arange(E, dtype=np.int32)))
    rowmap2 = rowmap.reshape(V, 1)
    if _TRI is None:
        _TRI = np.ascontiguousarray(np.triu(np.ones((L, L), dtype=np.float32)))

    if col_identity:
        nc = _get_module()
    else:
        # safety fallback (never taken in this environment): bake the column
        # permutation into a separate token-branch table
        nc = _get_module(dims_key="two_tables", dims={"TWO_TABLES": True})
        wte_tok = np.ascontiguousarray(wte[:, colmap])

    in_maps = []
    for m in range(N_CORES):
        sl = slice(m * B_LOC, (m + 1) * B_LOC)
        im = {
            "logits": logits[sl].reshape(B_LOC * L, V),
            "gumbel": gumbel[sl].reshape(B_LOC * L, V),
            "mask": mask[sl].reshape(B_LOC * L, 1),
            "psg": psg[sl].reshape(B_LOC * L, 1),
            "wte": wte,
            "rowmap": rowmap2,
            "tri": _TRI,
            "chain": np.zeros((L, 8), np.float32),
        }
        if not col_identity:
            im["wte_tok"] = wte_tok
        in_maps.append(im)

    global LAST_RESULT
    try:
        LAST_RESULT = run_bass_kernel_spmd(nc, in_maps, list(range(N_CORES)), trace=TRACE)
    except Exception:
        # the axon-relayed device occasionally reports a transient
        # NRT_EXEC_UNIT_UNRECOVERABLE on the first execution after long
        # sessions; a straight re-run recovers it
        import time as _time

        _time.sleep(2.0)
        LAST_RESULT = run_bass_kernel_spmd(nc, in_maps, list(range(N_CORES)), trace=TRACE)
    res = LAST_RESULT.results
    out = np.concatenate(
        [res[m]["out"].reshape(B_LOC, L, E) for m in range(N_CORES)], axis=0
    )
    return out
